# revision 1
# baseline (speedup 1.0000x reference)
"""Causal self-attention (B=4, T=2048, C=1024, H=16) on 8 TRN2 NeuronCores.

Sharding: data-parallel over B (4) x tensor-parallel over heads (2 halves of 8
heads). Core c handles batch c//2, heads 8*(c%2) .. 8*(c%2)+8. Each core runs
the full pipeline for its (batch, head-half); the host sums core pairs and
transposes.

Structure (chosen to minimise TensorE row-streaming cost):
- QKV projection in fp8(e4m3) hi/lo error-compensated DoubleRow matmuls:
  x = x_hi + x_lo, w = w_hi + w_lo; x@w ~ x_hi w_hi + x_hi w_lo + x_lo w_hi.
  DoubleRow packs two 128-row k-tiles per instruction.
- Scores S^T[k,q] in fp16 with block-causal skipping (128-row x 128-col
  granularity): only lower-triangular blocks are computed/exp'd.
- Exp on the Act engine; diagonal 128x128 triangles masked on GPSIMD.
- PV with the probability block as the *stationary* operand: out[q,65] per
  128-q-block (65th vaug column of ones gives softmax denominators), so each
  accumulation step streams only 65 rows.
- Per-q normalisation via DVE reciprocal + tensor_scalar broadcast.
- y[q,d] head pairs transposed on the PE (128x128, fp16) so the output
  projection contracts 128 rows per step (4 steps instead of 8).
- Output stored fp16; host sums head-half pairs in f32.
"""

import sys

if "/opt/trn_rl_repo" not in sys.path:
    sys.path.insert(0, "/opt/trn_rl_repo")

from contextlib import ExitStack

import numpy as np

import concourse.tile as tile
from concourse import bacc, mybir
from concourse.masks import make_identity

F32 = mybir.dt.float32
FP16 = mybir.dt.float16
FP8 = mybir.dt.float8e4
DR = mybir.MatmulPerfMode.DoubleRow
EXP = mybir.ActivationFunctionType.Exp

B, T, C, H = 4, 2048, 1024, 16
HL = 8  # heads per core
HD = 64  # head dim
CL = HL * HD  # local width (512)
W3 = 3 * CL  # qkv local col count (1536)
NT = T // 512  # 4 q-chunks of 512
NK2 = 4  # fp8 k-tile pairs over C (256 each)
NTT = T // 128  # 16 t-blocks of 128
# fp8 range fix: lo-parts of x (~2%) and w (~0.08%) underflow e4m3 subnormals
# (min 2^-9); scale operands up on the host, descale in the PSUM->SBUF copy.
FP8_SCALE_X = 8.0
FP8_SCALE_W = 64.0
FP8_DESCALE = 1.0 / (FP8_SCALE_X * FP8_SCALE_W)


def build_nc():
    nc = bacc.Bacc(None)

    xhh_d = nc.declare_dram_parameter("xhh", [NK2, 128, 2, T], FP8, isOutput=False)
    xll_d = nc.declare_dram_parameter("xll", [NK2, 128, 2, T], FP8, isOutput=False)
    whh_d = nc.declare_dram_parameter("whh", [NK2, 128, 2, W3], FP8, isOutput=False)
    wll_d = nc.declare_dram_parameter("wll", [NK2, 128, 2, W3], FP8, isOutput=False)
    wproj_d = nc.declare_dram_parameter("wproj", [4, 128, C], FP16, isOutput=False)
    bqk_d = nc.declare_dram_parameter("bqk", [128, 8], F32, isOutput=False)
    bv_d = nc.declare_dram_parameter("bv", [128, CL], F32, isOutput=False)
    bproj_d = nc.declare_dram_parameter("bproj", [128, 8], F32, isOutput=False)
    outT_d = nc.declare_dram_parameter("outT", [8, 128, T], FP16, isOutput=True)

    with tile.TileContext(nc) as tc, ExitStack() as ctx:
        persist = ctx.enter_context(tc.tile_pool(name="persist", bufs=1))
        # q^T / k^T blocks: nn 0..3 = q cols, 4..7 = k cols; [col128, T]
        qkT = [persist.tile([128, T], FP16, tag=f"qkT{nn}", name=f"qkT{nn}") for nn in range(8)]
        # v (+ ones col) per 128-t-block: [t128, head, 65]
        vaug = [persist.tile([128, HL, 65], FP16, tag=f"vaug{j}", name=f"vaug{j}") for j in range(NTT)]
        # y head-pairs [q128, qblock, dpair]; transposed copy [dpair, T]
        y_pair = [persist.tile([128, NTT, 128], FP16, tag=f"yp{hp}", name=f"yp{hp}") for hp in range(4)]
        ytp = [persist.tile([128, T], FP16, tag=f"ytp{hp}", name=f"ytp{hp}") for hp in range(4)]
        wproj_sb = [persist.tile([128, C], FP16, tag=f"wproj{hp}", name=f"wproj{hp}") for hp in range(4)]
        xhh = [persist.tile([128, 2, T], FP8, tag=f"xhh{k}", name=f"xhh{k}") for k in range(NK2)]
        xll = [persist.tile([128, 2, T], FP8, tag=f"xll{k}", name=f"xll{k}") for k in range(NK2)]
        whh = [persist.tile([128, 2, W3], FP8, tag=f"whh{k}", name=f"whh{k}") for k in range(NK2)]
        wll = [persist.tile([128, 2, W3], FP8, tag=f"wll{k}", name=f"wll{k}") for k in range(NK2)]
        bqk_sb = persist.tile([128, 8], F32, tag="bqk")
        bv_sb = persist.tile([128, CL], F32, tag="bv")
        bproj_sb = persist.tile([128, 8], F32, tag="bproj")
        ident = persist.tile([128, 128], FP16, tag="ident")

        # input DMAs, roughly in first-use order
        nc.sync.dma_start(bqk_sb[:], bqk_d[:])
        nc.sync.dma_start(bv_sb[:], bv_d[:])
        for k in range(NK2):
            nc.sync.dma_start(whh[k][:], whh_d[k])
            nc.sync.dma_start(wll[k][:], wll_d[k])
        for k in range(NK2):
            nc.sync.dma_start(xhh[k][:, :, 0:512], xhh_d[k][:, :, 0:512])
            nc.sync.dma_start(xll[k][:, :, 0:512], xll_d[k][:, :, 0:512])
        for k in range(NK2):
            nc.sync.dma_start(xhh[k][:, :, 512:T], xhh_d[k][:, :, 512:T])
            nc.sync.dma_start(xll[k][:, :, 512:T], xll_d[k][:, :, 512:T])
        for hp in range(4):
            nc.sync.dma_start(wproj_sb[hp][:], wproj_d[hp])
        nc.sync.dma_start(bproj_sb[:], bproj_d[:])
        make_identity(nc, ident)
        for j in range(NTT):
            nc.vector.memset(vaug[j][:, :, 64], 1.0)

        with (
            tc.tile_pool(name="work", bufs=1) as work,
            tc.tile_pool(name="ps", bufs=1, space="PSUM") as ps,
        ):

            def pt_tile():
                return work.tile([128, 1024], FP16, tag="pt", bufs=24, name="pt")

            def fp8_group(psum_region, stat_hh, stat_ll, mov_hh, mov_ll):
                """12 DoubleRow matmuls: hi*hi + hi*lo + lo*hi over 8 k-tiles."""
                n = 0
                for stat, mov in (
                    (stat_hh, mov_hh),
                    (stat_hh, mov_ll),
                    (stat_ll, mov_hh),
                ):
                    for k in range(NK2):
                        nc.tensor.matmul(
                            psum_region,
                            stat[k],
                            mov[k],
                            start=(n == 0),
                            stop=(n == 3 * NK2 - 1),
                            perf_mode=DR,
                        )
                        n += 1

            def v_unit(j):
                """v projection for t-block j -> vaug[j] (+bias).

                The first four (prologue) units use the otherwise-idle scores
                ring so they double-buffer instead of serialising on po."""
                if j < 4:
                    p_vt = ps.tile([128, 1024], F32, tag="pp", bufs=2, name="p_vt")
                    p_v = p_vt[:, 0:512]
                else:
                    p_v = ps.tile([128, 512], F32, tag="po", bufs=1)
                vs = slice(2 * CL, 3 * CL)
                bs = slice(j * 128, (j + 1) * 128)
                fp8_group(
                    p_v[:],
                    [xhh[k][:, :, bs] for k in range(NK2)],
                    [xll[k][:, :, bs] for k in range(NK2)],
                    [whh[k][:, :, vs] for k in range(NK2)],
                    [wll[k][:, :, vs] for k in range(NK2)],
                )
                nc.vector.scalar_tensor_tensor(
                    vaug[j][:, :, 0:64],
                    p_v[:].rearrange("p (h c) -> p h c", h=HL),
                    FP8_DESCALE,
                    bv_sb[:].rearrange("p (h c) -> p h c", h=HL),
                    mybir.AluOpType.mult,
                    mybir.AluOpType.add,
                )

            def qk_unit(nn, tcp, halves=(0, 1)):
                """q^T/k^T col-block nn for t-chunks 2*tcp+halves -> qkT[nn]."""
                p_qk = ps.tile([128, 1024], F32, tag="pp", bufs=2)
                ws = slice(nn * 128, (nn + 1) * 128)
                for half in halves:
                    ts_ = slice((2 * tcp + half) * 512, (2 * tcp + half + 1) * 512)
                    fp8_group(
                        p_qk[:, half * 512 : half * 512 + 512],
                        [whh[k][:, :, ws] for k in range(NK2)],
                        [wll[k][:, :, ws] for k in range(NK2)],
                        [xhh[k][:, :, ts_] for k in range(NK2)],
                        [xll[k][:, :, ts_] for k in range(NK2)],
                    )
                    nc.vector.tensor_scalar(
                        qkT[nn][
                            :,
                            (2 * tcp + half) * 512 : (2 * tcp + half + 1) * 512,
                        ],
                        p_qk[:, half * 512 : half * 512 + 512],
                        FP8_DESCALE,
                        bqk_sb[:, nn : nn + 1],
                        mybir.AluOpType.mult,
                        mybir.AluOpType.add,
                    )

            def att_scores_pieces(h, c, pts, mts):
                """Score pieces for (h, c): each closure emits one psum's
                matmuls + exp; the last also emits the 4 diagonal masks.
                Emitting pieces with other PE work woven between them keeps
                the in-order PE stream from stalling on the 2-deep score
                psum ring (which is paced by Act's exp throughput)."""
                poff = (h % 2) * 64
                kt = qkT[4 + h // 2]
                qt = qkT[h // 2]
                qs = slice(c * 512, (c + 1) * 512)
                pieces = []

                def full_pair(jp):
                    p_s = ps.tile([128, 1024], F32, tag="pp", bufs=2)
                    for half in range(2):
                        j = 2 * jp + half
                        nc.tensor.matmul(
                            p_s[:, half * 512 : half * 512 + 512],
                            kt[poff : poff + 64, j * 128 : (j + 1) * 128],
                            qt[poff : poff + 64, qs],
                            start=True,
                            stop=True,
                        )
                    pt = pt_tile()
                    nc.scalar.activation(pt[:], p_s[:], EXP)
                    pts.append(pt)

                def partial(pp_i):
                    p_s = ps.tile([128, 1024], F32, tag="pp", bufs=2)
                    off = 0
                    for half in range(2):
                        ti = 2 * pp_i + half
                        w = 512 - 128 * ti
                        j = 4 * c + ti
                        nc.tensor.matmul(
                            p_s[:, off : off + w],
                            kt[poff : poff + 64, j * 128 : (j + 1) * 128],
                            qt[poff : poff + 64, c * 512 + 128 * ti : (c + 1) * 512],
                            start=True,
                            stop=True,
                        )
                        off += w
                    pt = pt_tile()
                    nc.scalar.activation(pt[:, 0:off], p_s[:, 0:off], EXP)
                    pts.append(pt)

                def masks():
                    # diag triangles of (ti0,ti1) sit at offsets 0/512 of the
                    # first partial tile, (ti2,ti3) at 0/256 of the second:
                    # batch each pair as one strided affine_select
                    for pp_i, astr in ((0, 512), (1, 256)):
                        pt = pts[2 * c + pp_i]
                        src = pt[:, 0 : 2 * astr].rearrange(
                            "p (a w) -> p a w", a=2
                        )[:, :, 0:128]
                        mt = work.tile(
                            [128, 2, 128], FP16, tag="mt", bufs=8, name="mt"
                        )
                        nc.gpsimd.affine_select(
                            mt[:],
                            src,
                            pattern=[[0, 2], [1, 128]],
                            compare_op=mybir.AluOpType.is_ge,
                            fill=0.0,
                            base=0,
                            channel_multiplier=-1,
                        )
                        mts.append(mt)

                for jp in range(2 * c):
                    pieces.append(lambda jp=jp: full_pair(jp))
                pieces.append(lambda: partial(0))

                def last():
                    partial(1)
                    masks()

                pieces.append(last)
                return pieces

            def pv_block(p_y, h, c, pts, mts, tis):
                for ti in tis:
                    i = 4 * c + ti
                    ys = slice(ti * 65, ti * 65 + 65)
                    for j in range(i + 1):
                        if j == i:
                            blk = mts[ti // 2][:, ti % 2, :]
                        elif j >= 4 * c:
                            tj = j - 4 * c
                            off = (
                                0 if tj % 2 == 0 else 512 - 128 * (tj - 1)
                            ) + 128 * (ti - tj)
                            blk = pts[2 * c + tj // 2][:, off : off + 128]
                        else:
                            blk = pts[j // 2][
                                :,
                                (j % 2) * 512 + 128 * ti : (j % 2) * 512
                                + 128 * ti
                                + 128,
                            ]
                        nc.tensor.matmul(
                            p_y[:, ys],
                            blk,
                            vaug[j][:, h, :],
                            start=(j == 0),
                            stop=(j == i),
                        )

            def pv_norm(p_y, h, c):
                hp, doff = h // 2, (h % 2) * 64
                r = work.tile([128, 4], F32, tag="r", bufs=8, name="r")
                nc.vector.reciprocal(r[:], p_y[:, 64::65])
                nc.vector.tensor_tensor(
                    y_pair[hp][:, 4 * c : 4 * c + 4, doff : doff + 64],
                    p_y[:].rearrange("p (t k) -> p t k", t=4)[:, :, 0:64],
                    r[:].rearrange("p t -> p t ()").to_broadcast([128, 4, 64]),
                    mybir.AluOpType.mult,
                )

            def transposes(hp, c):
                p_t = ps.tile([128, 4, 128], FP16, tag="ptr", bufs=1)
                for ti in range(4):
                    qb = 4 * c + ti
                    nc.tensor.transpose(p_t[:, ti, :], y_pair[hp][:, qb, :], ident[:])
                nc.vector.tensor_copy(
                    ytp[hp][:, c * 512 : (c + 1) * 512].rearrange(
                        "p (a w) -> p a w", a=4
                    ),
                    p_t[:],
                )

            obig = {}

            def proj_unit(c, co):
                if co == 0:
                    obig[c] = work.tile(
                        [128, 8, 512], FP16, tag="os", bufs=1, name="os"
                    )
                o_big = obig[c]
                p_o = ps.tile([128, 512], F32, tag="po", bufs=1)
                for hp in range(4):
                    nc.tensor.matmul(
                        p_o[:],
                        wproj_sb[hp][:, co * 128 : (co + 1) * 128],
                        ytp[hp][:, c * 512 : (c + 1) * 512],
                        start=(hp == 0),
                        stop=(hp == 3),
                    )
                nc.vector.tensor_scalar_add(
                    o_big[:, co, :], p_o[:], bproj_sb[:, co : co + 1]
                )
                if co == 7:
                    nc.sync.dma_start(
                        outT_d[:, :, c * 512 : (c + 1) * 512].rearrange(
                            "co p t -> p co t"
                        ),
                        o_big[:],
                    )

            # ---------- cooperative piece scheduler ----------
            pv_q = []  # PV/norm pieces (priority: frees psum + pt rings)
            qkv_fifo = []  # ('qk'|'v', idx, closure) remaining qkv halves
            proj_q = []  # proj closures
            pending = []  # units whose scores are emitted, PV not yet
            done = set()
            transp_done = {}

            def pump(n):
                for _ in range(n):
                    if pv_q:
                        pv_q.pop(0)()
                    elif qkv_fifo:
                        qkv_fifo.pop(0)[2]()
                    elif proj_q:
                        proj_q.pop(0)()
                    else:
                        break

            def drain_qkv(kind, jmax):
                keep = []
                for item in qkv_fifo:
                    if item[0] == kind and item[1] <= jmax:
                        item[2]()
                    else:
                        keep.append(item)
                qkv_fifo[:] = keep

            def flush_one():
                c_, h_, pts_, mts_ = pending.pop(0)
                drain_qkv("v", 4 * c_ + 3)
                p_y = ps.tile([128, 260], F32, tag="pv", bufs=2)
                pv_q.append(lambda: pv_block(p_y, h_, c_, pts_, mts_, (0, 1)))

                def second():
                    pv_block(p_y, h_, c_, pts_, mts_, (2, 3))
                    pv_norm(p_y, h_, c_)
                    done.add((c_, h_))
                    if (c_, h_ ^ 1) in done:
                        transposes(h_ // 2, c_)
                        transp_done[c_] = transp_done.get(c_, 0) + 1
                        if transp_done[c_] == 4:
                            proj_q.extend(
                                (lambda c2=c_, co=co: proj_unit(c2, co))
                                for co in range(8)
                            )

                pv_q.append(second)

            def emit_unit(c, h):
                if c >= 2:
                    drain_qkv("qk", 99)
                pts, mts = [], []
                for piece in att_scores_pieces(h, c, pts, mts):
                    piece()
                    pump(1)
                pending.append((c, h, pts, mts))
                if len(pending) > 2:
                    flush_one()

            # ---- prologue: v for c0, qk tcp0 pairs with c0 units woven in ----
            for j in range(4):
                v_unit(j)
            for j in range(4, 8):
                qkv_fifo.append(("v", j, lambda j=j: v_unit(j)))
            for pair in range(4):
                qkv_fifo.append(
                    ("qk", 50 + pair, lambda nn=4 + pair: qk_unit(nn, 1))
                )
                qkv_fifo.append(("qk", 50 + pair, lambda nn=pair: qk_unit(nn, 1)))
            for j in range(8, 16):
                qkv_fifo.append(("v", j, lambda j=j: v_unit(j)))

            # first q/k pair per-512-chunk so Act's first exp starts as
            # soon as the tc0 x tiles land
            qk_unit(4, 0, halves=(0,))
            qk_unit(0, 0, halves=(0,))
            for pair in range(4):
                if pair > 0:
                    qk_unit(4 + pair, 0)
                    qk_unit(pair, 0)
                emit_unit(0, 2 * pair)
                if pair == 0:
                    qk_unit(4, 0, halves=(1,))
                    qk_unit(0, 0, halves=(1,))
                emit_unit(0, 2 * pair + 1)

            # c1 next (drains qkv), then c3 (Act-heaviest) zipped 2:1 with c2
            unit_order = [(1, h) for h in range(HL)]
            i2 = i3 = 0
            for grp in range(4):
                unit_order += [(3, i3), (3, i3 + 1), (2, i2)]
                i3 += 2
                i2 += 1
            unit_order += [(2, h) for h in range(4, 8)]

            for c, h in unit_order:
                emit_unit(c, h)
            while pending:
                flush_one()
                pump(4)
            while pv_q or qkv_fifo or proj_q:
                pump(8)

    nc.compile()
    return nc


# ---------------------------------------------------------------------------
# host side
# ---------------------------------------------------------------------------

_CACHE = {}


def _get_runner():
    if "runner" in _CACHE:
        return _CACHE["runner"]

    import jax
    from jax.experimental.shard_map import shard_map
    from jax.sharding import Mesh, PartitionSpec

    from concourse.bass2jax import (
        _bass_exec_p,
        install_neuronx_cc_hook,
        partition_id_tensor,
    )

    install_neuronx_cc_hook()
    nc = build_nc()
    n_cores = 8

    partition_name = nc.partition_id_tensor.name if nc.partition_id_tensor else None
    in_names = []
    out_names = []
    out_avals = []
    for alloc in nc.m.functions[0].allocations:
        if not isinstance(alloc, mybir.MemoryLocationSet):
            continue
        name = alloc.memorylocations[0].name
        if alloc.kind == "ExternalInput":
            if name != partition_name:
                in_names.append(name)
        elif alloc.kind == "ExternalOutput":
            out_names.append(name)
            out_avals.append(
                jax.core.ShapedArray(tuple(alloc.tensor_shape), mybir.dt.np(alloc.dtype))
            )
    n_params = len(in_names)
    all_names = in_names + out_names
    if partition_name is not None:
        all_names = all_names + [partition_name]

    def _body(*args):
        operands = list(args)
        if partition_name is not None:
            operands.append(partition_id_tensor())
        outs = _bass_exec_p.bind(
            *operands,
            out_avals=tuple(out_avals),
            in_names=tuple(all_names),
            out_names=tuple(out_names),
            lowering_input_output_aliases=(),
            sim_require_finite=True,
            sim_require_nnan=True,
            nc=nc,
        )
        return tuple(outs)

    devices = jax.devices()[:n_cores]
    mesh = Mesh(np.asarray(devices), ("core",))
    n_outs = len(out_names)
    fn = jax.jit(
        shard_map(
            _body,
            mesh=mesh,
            in_specs=(PartitionSpec("core"),) * (n_params + n_outs),
            out_specs=(PartitionSpec("core"),) * n_outs,
            check_rep=False,
        ),
        keep_unused=True,
    )

    runner = {
        "fn": fn,
        "in_names": in_names,
        "out_names": out_names,
        "out_avals": out_avals,
        "n_cores": n_cores,
        "jax": jax,
    }
    _CACHE["runner"] = runner
    return runner


def _pack_pairs(a):
    """[C, N] -> [NK2, 128, 2, N]: row 256*k + 128*s + p -> [k, p, s]."""
    n = a.shape[1]
    return np.ascontiguousarray(
        a.reshape(NK2, 2, 128, n).transpose(0, 2, 1, 3)
    )


def _hi_lo(a):
    from ml_dtypes import float8_e4m3

    hi = a.astype(float8_e4m3)
    lo = (a - hi.astype(np.float32)).astype(float8_e4m3)
    return hi, lo


def _prepare_in_maps(x, w_attn, b_attn, w_proj, b_proj):
    x = np.asarray(x, dtype=np.float32)
    w_attn = np.asarray(w_attn, dtype=np.float32)
    b_attn = np.asarray(b_attn, dtype=np.float32)
    w_proj = np.asarray(w_proj, dtype=np.float32)
    b_proj = np.asarray(b_proj, dtype=np.float32)

    in_maps = []
    for core in range(8):
        b = core // 2
        c0 = CL * (core % 2)

        xT = np.ascontiguousarray(x[b].T) * FP8_SCALE_X  # [C, T] f32
        x_hi, x_lo = _hi_lo(xT)
        xhh = _pack_pairs(x_hi)
        xll = _pack_pairs(x_lo)

        w_all = np.concatenate(
            [
                w_attn[:, c0 : c0 + CL] * 0.125,
                w_attn[:, C + c0 : C + c0 + CL],
                w_attn[:, 2 * C + c0 : 2 * C + c0 + CL],
            ],
            axis=1,
        )  # [C, 1536]
        w_all = w_all * FP8_SCALE_W
        w_hi, w_lo = _hi_lo(w_all)
        whh = _pack_pairs(w_hi)
        wll = _pack_pairs(w_lo)

        wproj = np.ascontiguousarray(
            w_proj[c0 : c0 + CL, :].reshape(4, 128, C)
        ).astype(np.float16)

        b_q = b_attn[c0 : c0 + CL] * 0.125
        b_k = b_attn[C + c0 : C + c0 + CL]
        bqk = np.ascontiguousarray(
            np.concatenate([b_q, b_k]).reshape(8, 128).T.astype(np.float32)
        )
        b_v = b_attn[2 * C + c0 : 2 * C + c0 + CL].astype(np.float32)
        bv = np.ascontiguousarray(np.broadcast_to(b_v[None, :], (128, CL)))
        if core % 2 == 0:
            bp = np.ascontiguousarray(b_proj.reshape(8, 128).T.astype(np.float32))
        else:
            bp = np.zeros((128, 8), dtype=np.float32)

        in_maps.append(
            {
                "xhh": xhh,
                "xll": xll,
                "whh": whh,
                "wll": wll,
                "wproj": wproj,
                "bqk": bqk,
                "bv": bv,
                "bproj": bp,
            }
        )
    return in_maps


def _run_device(in_maps):
    r = _get_runner()
    jax = r["jax"]
    n = r["n_cores"]
    per_core = [[np.asarray(m[name]) for name in r["in_names"]] for m in in_maps]
    concat_in = [
        np.concatenate([per_core[c][i] for c in range(n)], axis=0)
        for i in range(len(r["in_names"]))
    ]
    concat_zero = [
        np.zeros((n * a.shape[0], *a.shape[1:]), a.dtype) for a in r["out_avals"]
    ]
    outs = r["fn"](*[jax.device_put(a) for a in concat_in + concat_zero])
    jax.block_until_ready(outs)
    (outT,) = [np.asarray(o) for o in outs]
    return outT.reshape(n, C, T)


def kernel(x, w_attn, b_attn, w_proj, b_proj):
    in_maps = _prepare_in_maps(x, w_attn, b_attn, w_proj, b_proj)
    outT = _run_device(in_maps)
    out = np.empty((B, T, C), dtype=np.float32)
    for b in range(B):
        out[b] = (outT[2 * b].astype(np.float32) + outT[2 * b + 1].astype(np.float32)).T
    return out



# revision 55
# speedup vs baseline: 1.2135x; 1.2135x over previous
"""Causal self-attention (B=4, T=2048, C=1024, H=16) on 8 TRN2 NeuronCores.

Sharding: data-parallel over B (4) x tensor-parallel over heads (2 halves of 8
heads). Core c handles batch c//2, heads 8*(c%2) .. 8*(c%2)+8. Each core runs
the full pipeline for its (batch, head-half); the host sums core pairs and
transposes.

Structure (chosen to minimise TensorE row-streaming cost):
- QKV projection in fp8(e4m3) hi/lo error-compensated DoubleRow matmuls:
  x = x_hi + x_lo, w = w_hi + w_lo; x@w ~ x_hi w_hi + x_hi w_lo + x_lo w_hi.
  DoubleRow packs two 128-row k-tiles per instruction.
- Scores S^T[k,q] in fp16 with block-causal skipping (128-row x 128-col
  granularity): only lower-triangular blocks are computed/exp'd.
- Exp on the Act engine; diagonal 128x128 triangles masked on GPSIMD.
- PV with the probability block as the *stationary* operand: out[q,65] per
  128-q-block (65th vaug column of ones gives softmax denominators), so each
  accumulation step streams only 65 rows.
- Per-q normalisation via DVE reciprocal + tensor_scalar broadcast.
- y[q,d] head pairs transposed via the DMA XBAR (dma_start_transpose), not
  the PE, freeing both TensorE cycles and a PSUM bank.
- Startup: inputs land via a few large partition-major DMAs (first/wrest/
  xrest); warm-up matmuls on a zero tile keep TensorE busy (and its clock
  ramping) until the first real operands arrive.
- A pacing scheduler co-simulates the Act engine's exp backlog and weaves
  qkv/pv/proj work between score pieces so TensorE never waits on exp.
- Output stored fp16; host sums head-half pairs in f32.
"""

import sys

if "/opt/trn_rl_repo" not in sys.path:
    sys.path.insert(0, "/opt/trn_rl_repo")

from contextlib import ExitStack

import numpy as np

import concourse.tile as tile
from concourse import bacc, mybir

F32 = mybir.dt.float32
FP16 = mybir.dt.float16
FP8 = mybir.dt.float8e4
DR = mybir.MatmulPerfMode.DoubleRow
EXP = mybir.ActivationFunctionType.Exp

B, T, C, H = 4, 2048, 1024, 16
HL = 8  # heads per core
HD = 64  # head dim
CL = HL * HD  # local width (512)
W3 = 3 * CL  # qkv local col count (1536)
NK2 = 4  # fp8 k-tile pairs over C (256 each)
NTT = T // 128  # 16 t-blocks of 128
# fp8 range fix: lo-parts of x (~2%) and w (~0.08%) underflow e4m3 subnormals
# (min 2^-9); scale operands up on the host, descale in the PSUM->SBUF copy.
FP8_SCALE_X = 8.0
FP8_SCALE_W = 64.0
FP8_DESCALE = 1.0 / (FP8_SCALE_X * FP8_SCALE_W)

# cost-model constants (TimelineSim) used by the pacing scheduler
PE_CYC = 1.0 / 2.4  # ns per cycle, warm
ACT_CYC = 1.0 / 1.2
ACT_BUBBLE = 370.0  # per-instruction SBUF access bubble on Act
ACT_LAT = 420.0  # psum-ready -> act-start latency (pipeline+sem)
N_WARM = 14  # warm-up matmuls before the first DMA lands

# fast-exp (Schraudolph, fp16 bit trick): exp(s) ~ bitcast_fp16(int16(s*FE_A
# + FE_B)). Sawtooth rel err ~1.8% rms, mean ratio 1.0 (calibrated); used on
# a minority of off-diagonal score pieces to offload exp work from the Act
# engine to DVE/GPSIMD. Softmax normalisation absorbs the common-mode part.
FE_A = float(np.log2(np.e) * 1024.0)  # 1477.3197
FE_B = 15301.0
I16 = mybir.dt.int16
OFFLOAD_CAP = 40  # max fast-exp'd full pieces (error budget)

import os as _os
N_WARM = int(_os.environ.get("K_NWARM", N_WARM))
OFFLOAD_CAP = int(_os.environ.get("K_OFFCAP", OFFLOAD_CAP))
_PT_LIVE_CAP = int(_os.environ.get("K_PTLIVE", 16))
_DVE_COST = float(_os.environ.get("K_DVECOST", 1650.0))
_ACT_LAT2 = float(_os.environ.get("K_ACTLAT", 420.0))
_RING_POST = float(_os.environ.get("K_RINGPOST", 300.0))
_PROJ_RDY = float(_os.environ.get("K_PROJRDY", 9000.0))


def build_nc():
    nc = bacc.Bacc(None)

    # DRAM layouts, partition-major. first: w q-cols nn0,nn1 + k-cols nn4,nn5
    # + x t0:512. wrest: w q nn2,nn3 + k nn6,nn7 + v cols. xrest: x t512:2048.
    first_hh_d = nc.declare_dram_parameter("first_hh", [128, NK2, 2, 1024], FP8, isOutput=False)
    first_ll_d = nc.declare_dram_parameter("first_ll", [128, NK2, 2, 1024], FP8, isOutput=False)
    wrest_hh_d = nc.declare_dram_parameter("wrest_hh", [128, NK2, 2, 1024], FP8, isOutput=False)
    wrest_ll_d = nc.declare_dram_parameter("wrest_ll", [128, NK2, 2, 1024], FP8, isOutput=False)
    xrest_hh_d = nc.declare_dram_parameter("xrest_hh", [128, NK2, 2, 1536], FP8, isOutput=False)
    xrest_ll_d = nc.declare_dram_parameter("xrest_ll", [128, NK2, 2, 1536], FP8, isOutput=False)
    # w_proj fp8 hi/lo, DoubleRow-packed over the 512 contract rows:
    # [p, ktile, s, co] with contract row = 256*ktile + 128*s + p
    wpjhh_d = nc.declare_dram_parameter("wpjhh", [128, 2, 2, C], FP8, isOutput=False)
    wpjll_d = nc.declare_dram_parameter("wpjll", [128, 2, 2, C], FP8, isOutput=False)
    bqk_d = nc.declare_dram_parameter("bqk", [128, 8], F32, isOutput=False)
    bv_d = nc.declare_dram_parameter("bv", [128, CL], F32, isOutput=False)
    bproj_d = nc.declare_dram_parameter("bproj", [128, 8], F32, isOutput=False)
    outT_d = nc.declare_dram_parameter("outT", [8, 128, T], FP16, isOutput=True)

    with tile.TileContext(nc) as tc, ExitStack() as ctx:
        persist = ctx.enter_context(tc.tile_pool(name="persist", bufs=1))
        first_hh = persist.tile([128, NK2, 2, 1024], FP8, tag="fhh")
        first_ll = persist.tile([128, NK2, 2, 1024], FP8, tag="fll")
        wrest_hh = persist.tile([128, NK2, 2, 1024], FP8, tag="wrhh")
        wrest_ll = persist.tile([128, NK2, 2, 1024], FP8, tag="wrll")
        xrest_hh = persist.tile([128, NK2, 2, 1536], FP8, tag="xrhh")
        xrest_ll = persist.tile([128, NK2, 2, 1536], FP8, tag="xrll")
        # q^T / k^T blocks: nn 0..3 = q cols, 4..7 = k cols; [col128, T]
        qkT = [persist.tile([128, T], FP16, tag=f"qkT{nn}", name=f"qkT{nn}") for nn in range(8)]
        # v (+ ones col) per 128-t-block: [t128, head, 65]
        vaug = [persist.tile([128, HL, 65], FP16, tag=f"vaug{j}", name=f"vaug{j}") for j in range(NTT)]
        # y head-pairs [q128, qblock, dpair]; transposed chunks live in
        # per-chunk ring tiles (see transpose_dma / split_chunk below)
        y_pair = [persist.tile([128, NTT, 128], FP16, tag=f"yp{hp}", name=f"yp{hp}") for hp in range(4)]
        wpjhh_sb = persist.tile([128, 2, 2, C], FP8, tag="wpjhh")
        wpjll_sb = persist.tile([128, 2, 2, C], FP8, tag="wpjll")
        bqk_sb = persist.tile([128, 8], F32, tag="bqk")
        bv_sb = persist.tile([128, CL], F32, tag="bv")
        bproj_sb = persist.tile([128, 8], F32, tag="bproj")
        warm = persist.tile([128, 512], FP16, tag="warm")

        def w_slice(nn, k):
            """Stationary w tile for qk col-block nn, fp8 pair k: (hh, ll)."""
            src_h, src_l, off = {
                0: (first_hh, first_ll, 0),
                1: (first_hh, first_ll, 128),
                4: (first_hh, first_ll, 256),
                5: (first_hh, first_ll, 384),
                2: (wrest_hh, wrest_ll, 0),
                3: (wrest_hh, wrest_ll, 128),
                6: (wrest_hh, wrest_ll, 256),
                7: (wrest_hh, wrest_ll, 384),
            }[nn]
            return (src_h[:, k, :, off : off + 128], src_l[:, k, :, off : off + 128])

        def x_span(k, t0, t1):
            """Moving x tile [128, 2, t1-t0] for fp8 pair k: (hh, ll)."""
            if t1 <= 512:
                return (
                    first_hh[:, k, :, 512 + t0 : 512 + t1],
                    first_ll[:, k, :, 512 + t0 : 512 + t1],
                )
            assert t0 >= 512
            return (
                xrest_hh[:, k, :, t0 - 512 : t1 - 512],
                xrest_ll[:, k, :, t0 - 512 : t1 - 512],
            )

        def wv_slice(k):
            return (wrest_hh[:, k, :, 512:1024], wrest_ll[:, k, :, 512:1024])

        # ---- warm-up + input DMAs ----
        nc.vector.memset(warm[:], 0.0)

        with (
            tc.tile_pool(name="work", bufs=1) as work,
            tc.tile_pool(name="ps", bufs=1, space="PSUM") as ps,
        ):
            # warm-up: one accumulation group — same-engine ordering only, so
            # the matmuls run back-to-back and ramp the PE clock while the
            # first input DMAs are in flight
            p_w = ps.tile([128, 512], F32, tag="po", bufs=2)
            for i in range(N_WARM):
                nc.tensor.matmul(
                    p_w[:], warm[:, 0:128], warm[:],
                    start=(i == 0), stop=(i == N_WARM - 1),
                )

            # input DMAs: big tensors alternate the SP/Act HWDGE queues in
            # landing order first (w qk nn0/1/4/5 + x chunk0), wrest (w qk
            # rest + v), x rest; small/late tensors go via the parallel
            # gpsimd SWDGE path so they don't hold up the HWDGE pipeline.
            nc.gpsimd.dma_start(bqk_sb[:], bqk_d[:])
            nc.gpsimd.dma_start(bv_sb[:], bv_d[:])
            nc.sync.dma_start(first_hh[:], first_hh_d[:])
            nc.scalar.dma_start(first_ll[:], first_ll_d[:])
            nc.sync.dma_start(wrest_hh[:], wrest_hh_d[:])
            nc.scalar.dma_start(wrest_ll[:], wrest_ll_d[:])
            s1 = slice(0, 512)
            nc.sync.dma_start(xrest_hh[:, :, :, s1], xrest_hh_d[:, :, :, s1])
            nc.scalar.dma_start(xrest_ll[:, :, :, s1], xrest_ll_d[:, :, :, s1])
            for cc in range(1, 3):
                s = slice(cc * 512, cc * 512 + 512)
                nc.sync.dma_start(xrest_hh[:, :, :, s], xrest_hh_d[:, :, :, s])
                nc.scalar.dma_start(xrest_ll[:, :, :, s], xrest_ll_d[:, :, :, s])
            nc.gpsimd.dma_start(wpjhh_sb[:], wpjhh_d[:])
            nc.gpsimd.dma_start(wpjll_sb[:], wpjll_d[:])
            nc.gpsimd.dma_start(bproj_sb[:], bproj_d[:])
            for j in range(NTT):
                nc.vector.memset(vaug[j][:, :, 64], 1.0)

            def pt_tile():
                return work.tile([128, 1024], FP16, tag="pt", bufs=22, name="pt")

            def fp8_group(psum_region, stats, movs):
                """12 DoubleRow matmuls: hi*hi + hi*lo + lo*hi over 8 k-tiles.

                stats/movs: lists over k of (hh, ll) AP pairs."""
                n = 0
                for si, mi in ((0, 0), (0, 1), (1, 0)):
                    for k in range(NK2):
                        nc.tensor.matmul(
                            psum_region,
                            stats[k][si],
                            movs[k][mi],
                            start=(n == 0),
                            stop=(n == 3 * NK2 - 1),
                            perf_mode=DR,
                        )
                        n += 1

            def v_unit(j):
                """v projection for t-block j -> vaug[j] (+bias)."""
                p_v = ps.tile([128, 512], F32, tag="po", bufs=2)
                fp8_group(
                    p_v[:],
                    [x_span(k, j * 128, (j + 1) * 128) for k in range(NK2)],
                    [wv_slice(k) for k in range(NK2)],
                )
                nc.vector.scalar_tensor_tensor(
                    vaug[j][:, :, 0:64],
                    p_v[:].rearrange("p (h c) -> p h c", h=HL),
                    FP8_DESCALE,
                    bv_sb[:].rearrange("p (h c) -> p h c", h=HL),
                    mybir.AluOpType.mult,
                    mybir.AluOpType.add,
                )

            def qk_half(nn, th):
                """q^T/k^T col-block nn for t-half th (512 wide) -> qkT[nn]."""
                p_qk = ps.tile([128, 512], F32, tag="po", bufs=2)
                ts0 = th * 512
                fp8_group(
                    p_qk[:],
                    [w_slice(nn, k) for k in range(NK2)],
                    [x_span(k, ts0, ts0 + 512) for k in range(NK2)],
                )
                nc.vector.tensor_scalar(
                    qkT[nn][:, ts0 : ts0 + 512],
                    p_qk[:],
                    FP8_DESCALE,
                    bqk_sb[:, nn : nn + 1],
                    mybir.AluOpType.mult,
                    mybir.AluOpType.add,
                )

            def att_scores_pieces(h, c, pts, mts):
                """Score pieces for (h, c): each closure emits one psum's
                matmuls + exp; the last also emits the 4 diagonal masks.
                Returns list of (pe_ns, act_ns, closure)."""
                poff = (h % 2) * 64
                kt = qkT[4 + h // 2]
                qt = qkT[h // 2]
                qs = slice(c * 512, (c + 1) * 512)
                pieces = []

                def full_pair(jp, sink):
                    p_s = ps.tile([128, 1024], F32, tag="pp", bufs=3)
                    for half in range(2):
                        j = 2 * jp + half
                        nc.tensor.matmul(
                            p_s[:, half * 512 : half * 512 + 512],
                            kt[poff : poff + 64, j * 128 : (j + 1) * 128],
                            qt[poff : poff + 64, qs],
                            start=True,
                            stop=True,
                        )
                    pt = pt_tile()
                    if sink == "act":
                        nc.scalar.activation(pt[:], p_s[:], EXP)
                    else:
                        eng = nc.vector if sink == "dve" else nc.gpsimd
                        eng.tensor_scalar(
                            pt[:].bitcast(I16),
                            p_s[:],
                            FE_A,
                            FE_B,
                            mybir.AluOpType.mult,
                            mybir.AluOpType.add,
                        )
                    pts.append(pt)

                def partial(pp_i):
                    p_s = ps.tile([128, 1024], F32, tag="pp", bufs=3)
                    off = 0
                    for half in range(2):
                        ti = 2 * pp_i + half
                        w = 512 - 128 * ti
                        j = 4 * c + ti
                        nc.tensor.matmul(
                            p_s[:, off : off + w],
                            kt[poff : poff + 64, j * 128 : (j + 1) * 128],
                            qt[poff : poff + 64, c * 512 + 128 * ti : (c + 1) * 512],
                            start=True,
                            stop=True,
                        )
                        off += w
                    pt = pt_tile()
                    nc.scalar.activation(pt[:, 0:off], p_s[:, 0:off], EXP)
                    pts.append(pt)

                def masks():
                    # diag triangles of (ti0,ti1) sit at offsets 0/512 of the
                    # first partial tile, (ti2,ti3) at 0/256 of the second:
                    # batch each pair as one strided affine_select
                    for pp_i, astr in ((0, 512), (1, 256)):
                        pt = pts[2 * c + pp_i]
                        src = pt[:, 0 : 2 * astr].rearrange(
                            "p (a w) -> p a w", a=2
                        )[:, :, 0:128]
                        mt = work.tile(
                            [128, 2, 128], FP16, tag="mt", bufs=8, name="mt"
                        )
                        nc.gpsimd.affine_select(
                            mt[:],
                            src,
                            pattern=[[0, 2], [1, 128]],
                            compare_op=mybir.AluOpType.is_ge,
                            fill=0.0,
                            base=0,
                            channel_multiplier=-1,
                        )
                        mts.append(mt)

                # full pieces may be routed to DVE fast-exp at emission time
                # (sink=None = scheduler's choice, capped by error budget)
                for jp in range(2 * c):
                    pieces.append(
                        (427, 1100, jp // 2, None,
                         (lambda sink, jp=jp: full_pair(jp, sink)))
                    )
                pieces.append((373, 995, c, "act", lambda sink: partial(0)))

                def last(sink):
                    partial(1)
                    masks()

                pieces.append((160, 568, c, "act", last))
                return pieces

            def pv_block(p_y, h, c, pts, mts, tis):
                for ti in tis:
                    i = 4 * c + ti
                    ys = slice(ti * 65, ti * 65 + 65)
                    for j in range(i + 1):
                        if j == i:
                            blk = mts[ti // 2][:, ti % 2, :]
                        elif j >= 4 * c:
                            tj = j - 4 * c
                            off = (
                                0 if tj % 2 == 0 else 512 - 128 * (tj - 1)
                            ) + 128 * (ti - tj)
                            blk = pts[2 * c + tj // 2][:, off : off + 128]
                        else:
                            blk = pts[j // 2][
                                :,
                                (j % 2) * 512 + 128 * ti : (j % 2) * 512
                                + 128 * ti
                                + 128,
                            ]
                        nc.tensor.matmul(
                            p_y[:, ys],
                            blk,
                            vaug[j][:, h, :],
                            start=(j == 0),
                            stop=(j == i),
                        )

            def pv_norm(p_y, h, c):
                hp, doff = h // 2, (h % 2) * 64
                r = work.tile([128, 4], F32, tag="r", bufs=8, name="r")
                nc.vector.reciprocal(r[:], p_y[:, 64::65])
                nc.vector.tensor_tensor(
                    y_pair[hp][:, 4 * c : 4 * c + 4, doff : doff + 64],
                    p_y[:].rearrange("p (t k) -> p t k", t=4)[:, :, 0:64],
                    r[:].rearrange("p t -> p t ()").to_broadcast([128, 4, 64]),
                    mybir.AluOpType.mult,
                )

            # per-chunk transposed-y ring tiles: ytp fp16 (transpose target,
            # split source) and fp8 hi/lo (DoubleRow proj moving operands)
            ytp_t = {}  # (c, hp) -> fp16 [128, 4, 128]
            yhl_t = {}  # (c, k) -> (hi [128, 2, 512], lo [128, 2, 512])

            def transpose_dma(hp, c):
                """Block-transpose y_pair[hp] chunk c via the DMA XBAR:
                in [q=128,(qb,d)=512] -> out [d=128][qb=4][q=128]."""
                yt = work.tile(
                    [128, 4, 128], FP16, tag=f"ytp{hp}", bufs=2, name="yt"
                )
                ytp_t[(c, hp)] = yt
                nc.sync.dma_start_transpose(yt[:], y_pair[hp][:, 4 * c : 4 * c + 4, :])

            def split_chunk(c):
                """fp16 y^T chunk -> fp8 hi/lo (scaled by FP8_SCALE_X) for the
                DoubleRow projection. op1 on DVE, op2 on GPSIMD."""
                for k in range(2):
                    yhi = work.tile(
                        [128, 2, 512], FP8, tag=f"yhi{k}", bufs=2, name="yhi"
                    )
                    ylo = work.tile(
                        [128, 2, 512], FP8, tag=f"ylo{k}", bufs=2, name="ylo"
                    )
                    yhl_t[(c, k)] = (yhi, ylo)
                    for s in range(2):
                        src = ytp_t[(c, 2 * k + s)][:].rearrange("p a w -> p (a w)")
                        nc.vector.tensor_scalar(
                            yhi[:, s, :],
                            src,
                            FP8_SCALE_X,
                            0.0,
                            mybir.AluOpType.mult,
                            mybir.AluOpType.add,
                        )
                        nc.vector.scalar_tensor_tensor(
                            ylo[:, s, :],
                            src,
                            FP8_SCALE_X,
                            yhi[:, s, :],
                            mybir.AluOpType.mult,
                            mybir.AluOpType.subtract,
                        )

            def proj_pair(c, cop):
                """Output projection for co pair (2*cop, 2*cop+1) of chunk c:
                fp8 DoubleRow hi/lo (3 passes x 2 ktiles), drained to a
                staging tile then DMA'd per co."""
                o_s = work.tile([128, 2, 512], FP16, tag="os", bufs=4, name="os")
                for half in range(2):
                    co = 2 * cop + half
                    cs = slice(co * 128, (co + 1) * 128)
                    p_o = ps.tile([128, 512], F32, tag="po", bufs=2)
                    n = 0
                    for wi, yi in ((0, 0), (0, 1), (1, 0)):
                        for k in range(2):
                            w_ap = (wpjhh_sb if wi == 0 else wpjll_sb)[:, k, :, cs]
                            y_ap = yhl_t[(c, k)][yi][:]
                            nc.tensor.matmul(
                                p_o[:],
                                w_ap,
                                y_ap,
                                start=(n == 0),
                                stop=(n == 5),
                                perf_mode=DR,
                            )
                            n += 1
                    # psum drains must be DVE (GPSIMD has no PSUM access)
                    nc.vector.tensor_scalar(
                        o_s[:, half, :],
                        p_o[:],
                        FP8_DESCALE,
                        bproj_sb[:, co : co + 1],
                        mybir.AluOpType.mult,
                        mybir.AluOpType.add,
                    )
                    # per-half DMA: each fires as soon as its drain is done,
                    # shortening the end-of-kernel chain (SP queue only —
                    # a sem-waiting DMA on the Act queue would block exp)
                    nc.sync.dma_start(
                        outT_d[co, :, c * 512 : (c + 1) * 512],
                        o_s[:, half, :],
                    )

            # ---------- pacing scheduler ----------
            # co-simulates Act's exp backlog (cost-model constants) and pumps
            # qkv/pv/proj filler between score pieces so the 3-deep score
            # psum ring never blocks TensorE on the Act engine.
            st = {"cursor": 0.0, "act_fin": 0.0, "ps_idx": 0, "pt_live": 0,
                  "offloaded": 0, "units_left": 32}
            # per-sink backlog model; dve/pool per-piece costs inflated for
            # their unmodelled other work (drains / masks)
            sink_fin = {"act": 0.0, "dve": 0.0, "pool": 0.0}
            sink_cost = {"dve": _DVE_COST, "pool": 2100.0}
            ring_free_at = [0.0, 0.0, 0.0]  # exp-finish per 'pp' ring slot
            qkv_q = []  # (('qk',nn,th)|('v',j), pe_ns, closure)
            pv_q = []  # (ready_cursor, pe_ns, closure)
            proj_q = []  # (ready_cursor, pe_ns, closure)
            reserve_q = []  # proj pairs held back for the endgame
            pending = []
            done = set()
            transp_done = {}

            def emit(pe_ns, closure):
                closure()
                st["cursor"] += pe_ns

            def pump_one():
                """Emit one filler item; returns False if nothing available.

                Ready pv first (frees psum + pt rings), then qkv (available
                early, hoardable), then ready proj; as a last resort pop the
                least-unready pv/proj item."""
                if pv_q and pv_q[0][0] <= st["cursor"]:
                    _, pe_ns, cl = pv_q.pop(0)
                    emit(pe_ns, cl)
                    return True
                if qkv_q:
                    _, pe_ns, cl = qkv_q.pop(0)
                    emit(pe_ns, cl)
                    return True
                if proj_q and proj_q[0][0] <= st["cursor"]:
                    _, pe_ns, cl = proj_q.pop(0)
                    emit(pe_ns, cl)
                    return True
                if st["units_left"] == 0 and reserve_q:
                    _, pe_ns, cl = reserve_q.pop(0)
                    emit(pe_ns, cl)
                    return True
                best = None
                for q in (pv_q, proj_q):
                    if q and (best is None or q[0][0] < best[0][0]):
                        best = (q[0], q)
                if best is not None:
                    item, q = best
                    q.remove(item)
                    emit(item[1], item[2])
                    return True
                return False

            def drain_qkv(pred):
                keep = []
                for item in qkv_q:
                    if pred(item[0]):
                        emit(item[1], item[2])
                    else:
                        keep.append(item)
                qkv_q[:] = keep

            def emit_piece(pe_ns, act_ns, sink, closure):
                # keep the count of live exp'd tiles below the pt ring depth
                # (each piece holds one pt tile until its PV consumes it)
                while st["pt_live"] > _PT_LIVE_CAP and pv_q:
                    _, pv_pe, pv_cl = pv_q.pop(0)
                    emit(pv_pe, pv_cl)
                if sink is None:
                    # offloadable full piece: route to DVE fast-exp when Act
                    # would finish it later than DVE and error budget remains
                    t0 = st["cursor"] + _ACT_LAT2
                    fin_act = max(sink_fin["act"], t0) + act_ns
                    fin_dve = max(sink_fin["dve"], t0) + sink_cost["dve"]
                    if st["offloaded"] < OFFLOAD_CAP and fin_act > fin_dve:
                        sink = "dve"
                        st["offloaded"] += 1
                    else:
                        sink = "act"
                # ensure the ring slot this piece will reuse has been drained
                # by its exp engine before TensorE reaches the matmuls
                slot = st["ps_idx"] % 3
                st["ps_idx"] += 1
                while ring_free_at[slot] > st["cursor"]:
                    if not pump_one():
                        break
                emit(pe_ns, lambda: closure(sink))
                st["pt_live"] += 1
                cost = act_ns if sink == "act" else sink_cost[sink]
                start = max(sink_fin[sink], st["cursor"] + _ACT_LAT2)
                sink_fin[sink] = start + cost
                if sink == "act":
                    st["act_fin"] = sink_fin["act"]
                # +300: exp drain (init/2) + sem propagation before the psum
                # bank is reusable by TensorE
                ring_free_at[slot] = sink_fin[sink] + _RING_POST

            def flush_one():
                c_, h_, pts_, mts_, act_fin_ = pending.pop(0)
                drain_qkv(lambda k: k[0] == "v" and k[1] <= 4 * c_ + 3)
                nsteps = sum(4 * c_ + ti + 1 for ti in range(4))

                def pv_all():
                    # single item: the shared po-ring slot is held from first
                    # matmul to the norm drain, so no other po user may be
                    # emitted in between (PE is in-order)
                    p_yf = ps.tile([128, 512], F32, tag="po", bufs=2, name="p_yf")
                    p_y = p_yf[:, 0:260]
                    pv_block(p_y, h_, c_, pts_, mts_, (0, 1))
                    pv_block(p_y, h_, c_, pts_, mts_, (2, 3))
                    pv_norm(p_y, h_, c_)
                    st["pt_live"] -= 2 * c_ + 2
                    done.add((c_, h_))
                    if (c_, h_ ^ 1) in done:
                        transpose_dma(h_ // 2, c_)
                        transp_done[c_] = transp_done.get(c_, 0) + 1
                        if transp_done[c_] == 4:
                            split_chunk(c_)
                            # proj waits for the transpose DMA + hi/lo split
                            # chain (+margin); hold back two c3 pairs for the
                            # final chunk's transpose/split window
                            rdy = st["cursor"] + _PROJ_RDY
                            for cop in range(4):
                                item = (
                                    rdy,
                                    2 * 6 * 512 * 0.5 * PE_CYC,
                                    (lambda c2=c_, cop=cop: proj_pair(c2, cop)),
                                )
                                proj_q.append(item)

                pv_q.append((act_fin_ + 600.0, nsteps * 65 * PE_CYC, pv_all))

            def emit_unit(c, h):
                # data deps: qt col-block h//2 for chunk c up front; kt
                # col-block 4+h//2 drained lazily per piece, weaving the qk
                # fill work between score pieces
                g = h // 2
                drain_qkv(lambda k: k[0] == "qk" and k[1] == g and k[2] == c)
                pts, mts = [], []
                for pe_ns, act_ns, kt_th, sink, piece in att_scores_pieces(
                    h, c, pts, mts
                ):
                    drain_qkv(
                        lambda k: k[0] == "qk"
                        and k[1] == 4 + g
                        and k[2] <= kt_th
                    )
                    emit_piece(pe_ns, act_ns, sink, piece)
                pending.append((c, h, pts, mts, st["act_fin"]))
                if len(pending) > 2:
                    flush_one()

            # ---- fill qkv queue in data-arrival order ----
            qk_order = [0, 4, 1, 5, 2, 6, 3, 7]
            for th in range(4):
                for nn in qk_order:
                    qkv_q.append(
                        (("qk", nn, th), 1280, lambda nn=nn, th=th: qk_half(nn, th))
                    )
                for j in range(4 * th, 4 * th + 4):
                    qkv_q.append((("v", j), 1280, lambda j=j: v_unit(j)))

            # interleave chunks so the exp-heavy c3/c2 units start as soon as
            # their x data lands and the Act demand is spread evenly; chunk
            # completion staggered (c0 < c1 < c3 < c2) so each chunk's proj
            # fills the next chunk's tail
            unit_order = [
                (0, 0), (0, 1), (0, 2), (0, 3), (0, 4), (0, 5),
                (0, 6), (0, 7), (1, 0), (1, 1), (3, 0), (3, 1),
                (1, 2), (1, 3), (2, 0), (2, 1), (3, 2), (3, 3),
                (1, 4), (1, 5), (2, 2), (2, 3), (3, 4), (3, 5),
                (1, 6), (1, 7), (2, 4), (2, 5), (3, 6), (3, 7),
                (2, 6), (2, 7),
            ]
            assert sorted(unit_order) == sorted(
                (c, h) for c in range(4) for h in range(HL)
            )

            for c, h in unit_order:
                emit_unit(c, h)
                st["units_left"] -= 1
            while pending:
                flush_one()
                for _ in range(4):
                    pump_one()
            proj_q.extend(reserve_q)
            reserve_q[:] = []
            while pv_q or proj_q or qkv_q:
                if not pump_one():
                    break

    nc.compile()
    return nc


# ---------------------------------------------------------------------------
# host side
# ---------------------------------------------------------------------------

_CACHE = {}


def _get_runner():
    if "runner" in _CACHE:
        return _CACHE["runner"]

    import jax
    from jax.experimental.shard_map import shard_map
    from jax.sharding import Mesh, PartitionSpec

    from concourse.bass2jax import (
        _bass_exec_p,
        install_neuronx_cc_hook,
        partition_id_tensor,
    )

    install_neuronx_cc_hook()
    nc = build_nc()
    n_cores = 8

    partition_name = nc.partition_id_tensor.name if nc.partition_id_tensor else None
    in_names = []
    out_names = []
    out_avals = []
    for alloc in nc.m.functions[0].allocations:
        if not isinstance(alloc, mybir.MemoryLocationSet):
            continue
        name = alloc.memorylocations[0].name
        if alloc.kind == "ExternalInput":
            if name != partition_name:
                in_names.append(name)
        elif alloc.kind == "ExternalOutput":
            out_names.append(name)
            out_avals.append(
                jax.core.ShapedArray(tuple(alloc.tensor_shape), mybir.dt.np(alloc.dtype))
            )
    n_params = len(in_names)
    all_names = in_names + out_names
    if partition_name is not None:
        all_names = all_names + [partition_name]

    def _body(*args):
        operands = list(args)
        if partition_name is not None:
            operands.append(partition_id_tensor())
        outs = _bass_exec_p.bind(
            *operands,
            out_avals=tuple(out_avals),
            in_names=tuple(all_names),
            out_names=tuple(out_names),
            lowering_input_output_aliases=(),
            sim_require_finite=True,
            sim_require_nnan=True,
            nc=nc,
        )
        return tuple(outs)

    devices = jax.devices()[:n_cores]
    mesh = Mesh(np.asarray(devices), ("core",))
    n_outs = len(out_names)
    fn = jax.jit(
        shard_map(
            _body,
            mesh=mesh,
            in_specs=(PartitionSpec("core"),) * (n_params + n_outs),
            out_specs=(PartitionSpec("core"),) * n_outs,
            check_rep=False,
        ),
        keep_unused=True,
    )

    runner = {
        "fn": fn,
        "in_names": in_names,
        "out_names": out_names,
        "out_avals": out_avals,
        "n_cores": n_cores,
        "jax": jax,
    }
    _CACHE["runner"] = runner
    return runner


def _pack_pairs(a):
    """[C, N] -> [128, NK2, 2, N]: row 256*k + 128*s + p -> [p, k, s]."""
    n = a.shape[1]
    return np.ascontiguousarray(
        a.reshape(NK2, 2, 128, n).transpose(2, 0, 1, 3)
    )


def _hi_lo(a):
    from ml_dtypes import float8_e4m3

    hi = a.astype(float8_e4m3)
    lo = (a - hi.astype(np.float32)).astype(float8_e4m3)
    return hi, lo


def _prepare_in_maps(x, w_attn, b_attn, w_proj, b_proj):
    x = np.asarray(x, dtype=np.float32)
    w_attn = np.asarray(w_attn, dtype=np.float32)
    b_attn = np.asarray(b_attn, dtype=np.float32)
    w_proj = np.asarray(w_proj, dtype=np.float32)
    b_proj = np.asarray(b_proj, dtype=np.float32)

    in_maps = []
    for core in range(8):
        b = core // 2
        c0 = CL * (core % 2)

        xT = np.ascontiguousarray(x[b].T) * FP8_SCALE_X  # [C, T] f32
        x_hi, x_lo = _hi_lo(xT)
        xp_h = _pack_pairs(x_hi)  # [128, 4, 2, T]
        xp_l = _pack_pairs(x_lo)

        w_all = np.concatenate(
            [
                w_attn[:, c0 : c0 + CL] * 0.125,
                w_attn[:, C + c0 : C + c0 + CL],
                w_attn[:, 2 * C + c0 : 2 * C + c0 + CL],
            ],
            axis=1,
        )  # [C, 1536] = q(512) | k(512) | v(512)
        w_all = w_all * FP8_SCALE_W
        w_hi, w_lo = _hi_lo(w_all)
        wp_h = _pack_pairs(w_hi)  # [128, 4, 2, 1536]
        wp_l = _pack_pairs(w_lo)

        def pack_first(wp, xp):
            # w cols nn0(q0:128), nn1(q128:256), nn4(k512:640), nn5(k640:768)
            # then x t0:512
            return np.ascontiguousarray(
                np.concatenate(
                    [
                        wp[:, :, :, 0:128],
                        wp[:, :, :, 128:256],
                        wp[:, :, :, 512:640],
                        wp[:, :, :, 640:768],
                        xp[:, :, :, 0:512],
                    ],
                    axis=3,
                )
            )

        def pack_wrest(wp):
            # nn2(q256:384), nn3(q384:512), nn6(k768:896), nn7(k896:1024), v
            return np.ascontiguousarray(
                np.concatenate(
                    [
                        wp[:, :, :, 256:384],
                        wp[:, :, :, 384:512],
                        wp[:, :, :, 768:896],
                        wp[:, :, :, 896:1024],
                        wp[:, :, :, 1024:1536],
                    ],
                    axis=3,
                )
            )

        first_hh = pack_first(wp_h, xp_h)
        first_ll = pack_first(wp_l, xp_l)
        wrest_hh = pack_wrest(wp_h)
        wrest_ll = pack_wrest(wp_l)
        xrest_hh = np.ascontiguousarray(xp_h[:, :, :, 512:T])
        xrest_ll = np.ascontiguousarray(xp_l[:, :, :, 512:T])

        # w_proj fp8 hi/lo, DR-packed: row 256k+128s+p -> [p, k, s, :]
        wpj = w_proj[c0 : c0 + CL, :] * FP8_SCALE_W
        wpj_hi, wpj_lo = _hi_lo(wpj)
        wpjhh = np.ascontiguousarray(
            wpj_hi.reshape(2, 2, 128, C).transpose(2, 0, 1, 3)
        )
        wpjll = np.ascontiguousarray(
            wpj_lo.reshape(2, 2, 128, C).transpose(2, 0, 1, 3)
        )

        b_q = b_attn[c0 : c0 + CL] * 0.125
        b_k = b_attn[C + c0 : C + c0 + CL]
        bqk = np.ascontiguousarray(
            np.concatenate([b_q, b_k]).reshape(8, 128).T.astype(np.float32)
        )
        b_v = b_attn[2 * C + c0 : 2 * C + c0 + CL].astype(np.float32)
        bv = np.ascontiguousarray(np.broadcast_to(b_v[None, :], (128, CL)))
        if core % 2 == 0:
            bp = np.ascontiguousarray(b_proj.reshape(8, 128).T.astype(np.float32))
        else:
            bp = np.zeros((128, 8), dtype=np.float32)

        in_maps.append(
            {
                "first_hh": first_hh,
                "first_ll": first_ll,
                "wrest_hh": wrest_hh,
                "wrest_ll": wrest_ll,
                "xrest_hh": xrest_hh,
                "xrest_ll": xrest_ll,
                "wpjhh": wpjhh,
                "wpjll": wpjll,
                "bqk": bqk,
                "bv": bv,
                "bproj": bp,
            }
        )
    return in_maps


def _run_device(in_maps):
    r = _get_runner()
    jax = r["jax"]
    n = r["n_cores"]
    per_core = [[np.asarray(m[name]) for name in r["in_names"]] for m in in_maps]
    concat_in = [
        np.concatenate([per_core[c][i] for c in range(n)], axis=0)
        for i in range(len(r["in_names"]))
    ]
    concat_zero = [
        np.zeros((n * a.shape[0], *a.shape[1:]), a.dtype) for a in r["out_avals"]
    ]
    outs = r["fn"](*[jax.device_put(a) for a in concat_in + concat_zero])
    jax.block_until_ready(outs)
    (outT,) = [np.asarray(o) for o in outs]
    return outT.reshape(n, C, T)


def kernel(x, w_attn, b_attn, w_proj, b_proj):
    in_maps = _prepare_in_maps(x, w_attn, b_attn, w_proj, b_proj)
    outT = _run_device(in_maps)
    out = np.empty((B, T, C), dtype=np.float32)
    for b in range(B):
        out[b] = (outT[2 * b].astype(np.float32) + outT[2 * b + 1].astype(np.float32)).T
    return out


# revision 78
# speedup vs baseline: 1.2454x; 1.0263x over previous
"""Causal self-attention (B=4, T=2048, C=1024, H=16) on 8 TRN2 NeuronCores.

Sharding: data-parallel over B (4) x tensor-parallel over heads (2 halves of 8
heads). Core c handles batch c//2, heads 8*(c%2) .. 8*(c%2)+8. Each core runs
the full pipeline for its (batch, head-half); the host sums core pairs and
transposes.

Structure (chosen to minimise TensorE row-streaming cost):
- QKV projection in fp8(e4m3) hi/lo error-compensated DoubleRow matmuls:
  x = x_hi + x_lo, w = w_hi + w_lo; x@w ~ x_hi w_hi + x_hi w_lo + x_lo w_hi.
  DoubleRow packs two 128-row k-tiles per instruction.
- Scores S^T[k,q] in fp16 with block-causal skipping (128-row x 128-col
  granularity): only lower-triangular blocks are computed/exp'd.
- Exp on the Act engine; diagonal 128x128 triangles masked on GPSIMD.
- PV with the probability block as the *stationary* operand: out[q,65] per
  128-q-block (65th vaug column of ones gives softmax denominators), so each
  accumulation step streams only 65 rows.
- Per-q normalisation via DVE reciprocal + tensor_scalar broadcast.
- y[q,d] head pairs transposed via the DMA XBAR (dma_start_transpose), not
  the PE, freeing both TensorE cycles and a PSUM bank.
- Startup: inputs land via a few large partition-major DMAs (first/wrest/
  xrest); warm-up matmuls on a zero tile keep TensorE busy (and its clock
  ramping) until the first real operands arrive.
- A pacing scheduler co-simulates the Act engine's exp backlog and weaves
  qkv/pv/proj work between score pieces so TensorE never waits on exp.
- Output stored fp16; host sums head-half pairs in f32.
"""

import sys

if "/opt/trn_rl_repo" not in sys.path:
    sys.path.insert(0, "/opt/trn_rl_repo")

from contextlib import ExitStack

import numpy as np

import concourse.tile as tile
from concourse import bacc, mybir

F32 = mybir.dt.float32
FP16 = mybir.dt.float16
FP8 = mybir.dt.float8e4
DR = mybir.MatmulPerfMode.DoubleRow
EXP = mybir.ActivationFunctionType.Exp

B, T, C, H = 4, 2048, 1024, 16
HL = 8  # heads per core
HD = 64  # head dim
CL = HL * HD  # local width (512)
W3 = 3 * CL  # qkv local col count (1536)
NK2 = 4  # fp8 k-tile pairs over C (256 each)
NTT = T // 128  # 16 t-blocks of 128
# fp8 range fix: lo-parts of x (~2%) and w (~0.08%) underflow e4m3 subnormals
# (min 2^-9); scale operands up on the host, descale in the PSUM->SBUF copy.
FP8_SCALE_X = 8.0
FP8_SCALE_W = 64.0
FP8_DESCALE = 1.0 / (FP8_SCALE_X * FP8_SCALE_W)

# cost-model constants (TimelineSim) used by the pacing scheduler
PE_CYC = 1.0 / 2.4  # ns per cycle, warm
ACT_CYC = 1.0 / 1.2
ACT_BUBBLE = 370.0  # per-instruction SBUF access bubble on Act
ACT_LAT = 420.0  # psum-ready -> act-start latency (pipeline+sem)
N_WARM = 14  # warm-up matmuls before the first DMA lands

# fast-exp (Schraudolph, fp16 bit trick): exp(s) ~ bitcast_fp16(int16(s*FE_A
# + FE_B)). Sawtooth rel err ~1.8% rms, mean ratio 1.0 (calibrated); used on
# a minority of off-diagonal score pieces to offload exp work from the Act
# engine to DVE/GPSIMD. Softmax normalisation absorbs the common-mode part.
FE_A = float(np.log2(np.e) * 1024.0)  # 1477.3197
FE_B = 15301.0
I16 = mybir.dt.int16
OFFLOAD_CAP = 40  # max fast-exp'd full pieces (error budget)

import os as _os
N_WARM = int(_os.environ.get("K_NWARM", N_WARM))
OFFLOAD_CAP = int(_os.environ.get("K_OFFCAP", 48))
_PT_LIVE_CAP = int(_os.environ.get("K_PTLIVE", 16))
_DVE_COST = float(_os.environ.get("K_DVECOST", 2000.0))
_ACT_LAT2 = float(_os.environ.get("K_ACTLAT", 420.0))
_RING_POST = float(_os.environ.get("K_RINGPOST", 650.0))
_PROJ_RDY = float(_os.environ.get("K_PROJRDY", 9000.0))
_TAIL_GATE = int(_os.environ.get("K_TAILGATE", 0))
_PEND = int(_os.environ.get("K_PEND", 2))
_OFF_BIAS = float(_os.environ.get("K_OFFBIAS", 0.0))


def build_nc():
    nc = bacc.Bacc(None)

    # DRAM layouts, partition-major. first: w q-cols nn0,nn1 + k-cols nn4,nn5
    # + x t0:512. wrest: w q nn2,nn3 + k nn6,nn7 + v cols. xrest: x t512:2048.
    first_hh_d = nc.declare_dram_parameter("first_hh", [128, NK2, 2, 1024], FP8, isOutput=False)
    first_ll_d = nc.declare_dram_parameter("first_ll", [128, NK2, 2, 1024], FP8, isOutput=False)
    wrest_hh_d = nc.declare_dram_parameter("wrest_hh", [128, NK2, 2, 1024], FP8, isOutput=False)
    wrest_ll_d = nc.declare_dram_parameter("wrest_ll", [128, NK2, 2, 1024], FP8, isOutput=False)
    xrest_hh_d = nc.declare_dram_parameter("xrest_hh", [128, NK2, 2, 1536], FP8, isOutput=False)
    xrest_ll_d = nc.declare_dram_parameter("xrest_ll", [128, NK2, 2, 1536], FP8, isOutput=False)
    # w_proj fp8 hi/lo, DoubleRow-packed over the 512 contract rows:
    # [p, ktile, s, co] with contract row = 256*ktile + 128*s + p
    wpjhh_d = nc.declare_dram_parameter("wpjhh", [128, 2, 2, C], FP8, isOutput=False)
    wpjll_d = nc.declare_dram_parameter("wpjll", [128, 2, 2, C], FP8, isOutput=False)
    bqk_d = nc.declare_dram_parameter("bqk", [128, 8], F32, isOutput=False)
    bv_d = nc.declare_dram_parameter("bv", [128, CL], F32, isOutput=False)
    bproj_d = nc.declare_dram_parameter("bproj", [128, 8], F32, isOutput=False)
    outT_d = nc.declare_dram_parameter("outT", [8, 128, T], FP16, isOutput=True)

    with tile.TileContext(nc) as tc, ExitStack() as ctx:
        persist = ctx.enter_context(tc.tile_pool(name="persist", bufs=1))
        first_hh = persist.tile([128, NK2, 2, 1024], FP8, tag="fhh")
        first_ll = persist.tile([128, NK2, 2, 1024], FP8, tag="fll")
        wrest_hh = persist.tile([128, NK2, 2, 1024], FP8, tag="wrhh")
        wrest_ll = persist.tile([128, NK2, 2, 1024], FP8, tag="wrll")
        xrest_hh = persist.tile([128, NK2, 2, 1536], FP8, tag="xrhh")
        xrest_ll = persist.tile([128, NK2, 2, 1536], FP8, tag="xrll")
        # q^T / k^T blocks: nn 0..3 = q cols, 4..7 = k cols; [col128, T]
        qkT = [persist.tile([128, T], FP16, tag=f"qkT{nn}", name=f"qkT{nn}") for nn in range(8)]
        # v (+ ones col) per 128-t-block: [t128, head, 65]
        vaug = [persist.tile([128, HL, 65], FP16, tag=f"vaug{j}", name=f"vaug{j}") for j in range(NTT)]
        # y head-pairs [q128, qblock, dpair]; transposed chunks live in
        # per-chunk ring tiles (see transpose_dma / split_chunk below)
        y_pair = [persist.tile([128, NTT, 128], FP16, tag=f"yp{hp}", name=f"yp{hp}") for hp in range(4)]
        wpjhh_sb = persist.tile([128, 2, 2, C], FP8, tag="wpjhh")
        wpjll_sb = persist.tile([128, 2, 2, C], FP8, tag="wpjll")
        bqk_sb = persist.tile([128, 8], F32, tag="bqk")
        bv_sb = persist.tile([128, CL], F32, tag="bv")
        bproj_sb = persist.tile([128, 8], F32, tag="bproj")
        warm = persist.tile([128, 512], FP16, tag="warm")

        def w_slice(nn, k):
            """Stationary w tile for qk col-block nn, fp8 pair k: (hh, ll)."""
            src_h, src_l, off = {
                0: (first_hh, first_ll, 0),
                1: (first_hh, first_ll, 128),
                4: (first_hh, first_ll, 256),
                5: (first_hh, first_ll, 384),
                2: (wrest_hh, wrest_ll, 0),
                3: (wrest_hh, wrest_ll, 128),
                6: (wrest_hh, wrest_ll, 256),
                7: (wrest_hh, wrest_ll, 384),
            }[nn]
            return (src_h[:, k, :, off : off + 128], src_l[:, k, :, off : off + 128])

        def x_span(k, t0, t1):
            """Moving x tile [128, 2, t1-t0] for fp8 pair k: (hh, ll)."""
            if t1 <= 512:
                return (
                    first_hh[:, k, :, 512 + t0 : 512 + t1],
                    first_ll[:, k, :, 512 + t0 : 512 + t1],
                )
            assert t0 >= 512
            return (
                xrest_hh[:, k, :, t0 - 512 : t1 - 512],
                xrest_ll[:, k, :, t0 - 512 : t1 - 512],
            )

        def wv_slice(k):
            return (wrest_hh[:, k, :, 512:1024], wrest_ll[:, k, :, 512:1024])

        # ---- warm-up + input DMAs ----
        nc.vector.memset(warm[:], 0.0)

        with (
            tc.tile_pool(name="work", bufs=1) as work,
            tc.tile_pool(name="ps", bufs=1, space="PSUM") as ps,
        ):
            # warm-up: one accumulation group — same-engine ordering only, so
            # the matmuls run back-to-back and ramp the PE clock while the
            # first input DMAs are in flight
            p_w = ps.tile([128, 512], F32, tag="po", bufs=2)
            for i in range(N_WARM):
                nc.tensor.matmul(
                    p_w[:], warm[:, 0:128], warm[:],
                    start=(i == 0), stop=(i == N_WARM - 1),
                )

            # input DMAs: big tensors alternate the SP/Act HWDGE queues in
            # landing order first (w qk nn0/1/4/5 + x chunk0), wrest (w qk
            # rest + v), x rest; small/late tensors go via the parallel
            # gpsimd SWDGE path so they don't hold up the HWDGE pipeline.
            nc.gpsimd.dma_start(bqk_sb[:], bqk_d[:])
            nc.gpsimd.dma_start(bv_sb[:], bv_d[:])
            nc.sync.dma_start(first_hh[:], first_hh_d[:])
            nc.scalar.dma_start(first_ll[:], first_ll_d[:])
            nc.sync.dma_start(wrest_hh[:], wrest_hh_d[:])
            nc.scalar.dma_start(wrest_ll[:], wrest_ll_d[:])
            s1 = slice(0, 512)
            nc.sync.dma_start(xrest_hh[:, :, :, s1], xrest_hh_d[:, :, :, s1])
            nc.scalar.dma_start(xrest_ll[:, :, :, s1], xrest_ll_d[:, :, :, s1])
            for cc in range(1, 3):
                s = slice(cc * 512, cc * 512 + 512)
                nc.sync.dma_start(xrest_hh[:, :, :, s], xrest_hh_d[:, :, :, s])
                nc.scalar.dma_start(xrest_ll[:, :, :, s], xrest_ll_d[:, :, :, s])
            nc.gpsimd.dma_start(wpjhh_sb[:], wpjhh_d[:])
            nc.gpsimd.dma_start(wpjll_sb[:], wpjll_d[:])
            nc.gpsimd.dma_start(bproj_sb[:], bproj_d[:])
            for j in range(NTT):
                nc.vector.memset(vaug[j][:, :, 64], 1.0)

            def pt_tile():
                return work.tile([128, 1024], FP16, tag="pt", bufs=22, name="pt")

            def fp8_group(psum_region, stats, movs):
                """12 DoubleRow matmuls: hi*hi + hi*lo + lo*hi over 8 k-tiles.

                stats/movs: lists over k of (hh, ll) AP pairs."""
                n = 0
                for si, mi in ((0, 0), (0, 1), (1, 0)):
                    for k in range(NK2):
                        nc.tensor.matmul(
                            psum_region,
                            stats[k][si],
                            movs[k][mi],
                            start=(n == 0),
                            stop=(n == 3 * NK2 - 1),
                            perf_mode=DR,
                        )
                        n += 1

            def v_unit(j):
                """v projection for t-block j -> vaug[j] (+bias)."""
                p_v = ps.tile([128, 512], F32, tag="po", bufs=2)
                fp8_group(
                    p_v[:],
                    [x_span(k, j * 128, (j + 1) * 128) for k in range(NK2)],
                    [wv_slice(k) for k in range(NK2)],
                )
                nc.vector.scalar_tensor_tensor(
                    vaug[j][:, :, 0:64],
                    p_v[:].rearrange("p (h c) -> p h c", h=HL),
                    FP8_DESCALE,
                    bv_sb[:].rearrange("p (h c) -> p h c", h=HL),
                    mybir.AluOpType.mult,
                    mybir.AluOpType.add,
                )

            def qk_half(nn, th):
                """q^T/k^T col-block nn for t-half th (512 wide) -> qkT[nn]."""
                p_qk = ps.tile([128, 512], F32, tag="po", bufs=2)
                ts0 = th * 512
                fp8_group(
                    p_qk[:],
                    [w_slice(nn, k) for k in range(NK2)],
                    [x_span(k, ts0, ts0 + 512) for k in range(NK2)],
                )
                nc.vector.tensor_scalar(
                    qkT[nn][:, ts0 : ts0 + 512],
                    p_qk[:],
                    FP8_DESCALE,
                    bqk_sb[:, nn : nn + 1],
                    mybir.AluOpType.mult,
                    mybir.AluOpType.add,
                )

            def att_scores_pieces(h, c, pts, mts):
                """Score pieces for (h, c): each closure emits one psum's
                matmuls + exp; the last also emits the 4 diagonal masks.
                Returns list of (pe_ns, act_ns, closure)."""
                poff = (h % 2) * 64
                kt = qkT[4 + h // 2]
                qt = qkT[h // 2]
                qs = slice(c * 512, (c + 1) * 512)
                pieces = []

                def full_pair(jp, sink):
                    p_s = ps.tile([128, 1024], F32, tag="pp", bufs=3)
                    for half in range(2):
                        j = 2 * jp + half
                        nc.tensor.matmul(
                            p_s[:, half * 512 : half * 512 + 512],
                            kt[poff : poff + 64, j * 128 : (j + 1) * 128],
                            qt[poff : poff + 64, qs],
                            start=True,
                            stop=True,
                        )
                    pt = pt_tile()
                    if sink == "act":
                        nc.scalar.activation(pt[:], p_s[:], EXP)
                    else:
                        eng = nc.vector if sink == "dve" else nc.gpsimd
                        eng.tensor_scalar(
                            pt[:].bitcast(I16),
                            p_s[:],
                            FE_A,
                            FE_B,
                            mybir.AluOpType.mult,
                            mybir.AluOpType.add,
                        )
                    pts.append(pt)

                def partial(pp_i):
                    p_s = ps.tile([128, 1024], F32, tag="pp", bufs=3)
                    off = 0
                    for half in range(2):
                        ti = 2 * pp_i + half
                        w = 512 - 128 * ti
                        j = 4 * c + ti
                        nc.tensor.matmul(
                            p_s[:, off : off + w],
                            kt[poff : poff + 64, j * 128 : (j + 1) * 128],
                            qt[poff : poff + 64, c * 512 + 128 * ti : (c + 1) * 512],
                            start=True,
                            stop=True,
                        )
                        off += w
                    pt = pt_tile()
                    nc.scalar.activation(pt[:, 0:off], p_s[:, 0:off], EXP)
                    pts.append(pt)

                def masks():
                    # diag triangles of (ti0,ti1) sit at offsets 0/512 of the
                    # first partial tile, (ti2,ti3) at 0/256 of the second:
                    # batch each pair as one strided affine_select
                    for pp_i, astr in ((0, 512), (1, 256)):
                        pt = pts[2 * c + pp_i]
                        src = pt[:, 0 : 2 * astr].rearrange(
                            "p (a w) -> p a w", a=2
                        )[:, :, 0:128]
                        mt = work.tile(
                            [128, 2, 128], FP16, tag="mt", bufs=8, name="mt"
                        )
                        nc.gpsimd.affine_select(
                            mt[:],
                            src,
                            pattern=[[0, 2], [1, 128]],
                            compare_op=mybir.AluOpType.is_ge,
                            fill=0.0,
                            base=0,
                            channel_multiplier=-1,
                        )
                        mts.append(mt)

                # full pieces may be routed to DVE fast-exp at emission time
                # (sink=None = scheduler's choice, capped by error budget)
                for jp in range(2 * c):
                    pieces.append(
                        (427, 1100, jp // 2, None,
                         (lambda sink, jp=jp: full_pair(jp, sink)))
                    )
                pieces.append((373, 995, c, "act", lambda sink: partial(0)))

                def last(sink):
                    partial(1)
                    masks()

                pieces.append((160, 568, c, "act", last))
                return pieces

            def pv_block(p_y, h, c, pts, mts, tis):
                for ti in tis:
                    i = 4 * c + ti
                    ys = slice(ti * 65, ti * 65 + 65)
                    for j in range(i + 1):
                        if j == i:
                            blk = mts[ti // 2][:, ti % 2, :]
                        elif j >= 4 * c:
                            tj = j - 4 * c
                            off = (
                                0 if tj % 2 == 0 else 512 - 128 * (tj - 1)
                            ) + 128 * (ti - tj)
                            blk = pts[2 * c + tj // 2][:, off : off + 128]
                        else:
                            blk = pts[j // 2][
                                :,
                                (j % 2) * 512 + 128 * ti : (j % 2) * 512
                                + 128 * ti
                                + 128,
                            ]
                        nc.tensor.matmul(
                            p_y[:, ys],
                            blk,
                            vaug[j][:, h, :],
                            start=(j == 0),
                            stop=(j == i),
                        )

            def pv_norm(p_y, h, c):
                hp, doff = h // 2, (h % 2) * 64
                r = work.tile([128, 4], F32, tag="r", bufs=8, name="r")
                nc.vector.reciprocal(r[:], p_y[:, 64::65])
                nc.vector.tensor_tensor(
                    y_pair[hp][:, 4 * c : 4 * c + 4, doff : doff + 64],
                    p_y[:].rearrange("p (t k) -> p t k", t=4)[:, :, 0:64],
                    r[:].rearrange("p t -> p t ()").to_broadcast([128, 4, 64]),
                    mybir.AluOpType.mult,
                )

            # per-chunk transposed-y ring tiles: ytp fp16 (transpose target,
            # split source) and fp8 hi/lo (DoubleRow proj moving operands)
            ytp_t = {}  # (c, hp) -> fp16 [128, 4, 128]
            yhl_t = {}  # (c, k) -> (hi [128, 2, 512], lo [128, 2, 512])

            def transpose_dma(hp, c):
                """Block-transpose y_pair[hp] chunk c via the DMA XBAR:
                in [q=128,(qb,d)=512] -> out [d=128][qb=4][q=128]."""
                yt = work.tile(
                    [128, 4, 128], FP16, tag=f"ytp{hp}", bufs=2, name="yt"
                )
                ytp_t[(c, hp)] = yt
                nc.sync.dma_start_transpose(yt[:], y_pair[hp][:, 4 * c : 4 * c + 4, :])

            def split_ktile(c, k):
                """fp16 y^T chunk (head-pair ktile k) -> fp8 hi/lo (scaled by
                FP8_SCALE_X) for the DoubleRow projection."""
                if True:
                    yhi = work.tile(
                        [128, 2, 512], FP8, tag=f"yhi{k}", bufs=2, name="yhi"
                    )
                    ylo = work.tile(
                        [128, 2, 512], FP8, tag=f"ylo{k}", bufs=2, name="ylo"
                    )
                    yhl_t[(c, k)] = (yhi, ylo)
                    for s in range(2):
                        src = ytp_t[(c, 2 * k + s)][:].rearrange("p a w -> p (a w)")
                        nc.vector.tensor_scalar(
                            yhi[:, s, :],
                            src,
                            FP8_SCALE_X,
                            0.0,
                            mybir.AluOpType.mult,
                            mybir.AluOpType.add,
                        )
                        nc.vector.scalar_tensor_tensor(
                            ylo[:, s, :],
                            src,
                            FP8_SCALE_X,
                            yhi[:, s, :],
                            mybir.AluOpType.mult,
                            mybir.AluOpType.subtract,
                        )

            def proj_pair(c, cop):
                """Output projection for co pair (2*cop, 2*cop+1) of chunk c:
                fp8 DoubleRow hi/lo (3 passes x 2 ktiles), drained to a
                staging tile then DMA'd per co."""
                o_s = work.tile([128, 2, 512], FP16, tag="os", bufs=4, name="os")
                for half in range(2):
                    co = 2 * cop + half
                    cs = slice(co * 128, (co + 1) * 128)
                    p_o = ps.tile([128, 512], F32, tag="po", bufs=2)
                    n = 0
                    # y_hi-only passes first: the first four matmuls can start
                    # before the (later) y_lo split lands
                    for wi, yi in ((0, 0), (1, 0), (0, 1)):
                        for k in range(2):
                            w_ap = (wpjhh_sb if wi == 0 else wpjll_sb)[:, k, :, cs]
                            y_ap = yhl_t[(c, k)][yi][:]
                            nc.tensor.matmul(
                                p_o[:],
                                w_ap,
                                y_ap,
                                start=(n == 0),
                                stop=(n == 5),
                                perf_mode=DR,
                            )
                            n += 1
                    # psum drains: DVE, except the last chunk's odd halves
                    # which go to the by-then-idle Act engine (GPSIMD has no
                    # PSUM access)
                    nc.vector.tensor_scalar(
                        o_s[:, half, :],
                        p_o[:],
                        FP8_DESCALE,
                        bproj_sb[:, co : co + 1],
                        mybir.AluOpType.mult,
                        mybir.AluOpType.add,
                    )
                    # per-half DMA: each fires as soon as its drain is done
                    # (SP queue only — a sem-waiting DMA on the Act queue
                    # would block exp)
                    nc.sync.dma_start(
                        outT_d[co, :, c * 512 : (c + 1) * 512],
                        o_s[:, half, :],
                    )

            # ---------- pacing scheduler ----------
            # co-simulates Act's exp backlog (cost-model constants) and pumps
            # qkv/pv/proj filler between score pieces so the 3-deep score
            # psum ring never blocks TensorE on the Act engine.
            st = {"cursor": 0.0, "act_fin": 0.0, "ps_idx": 0, "pt_live": 0,
                  "offloaded": 0, "units_left": 32}
            # per-sink backlog model; dve/pool per-piece costs inflated for
            # their unmodelled other work (drains / masks)
            sink_fin = {"act": 0.0, "dve": 0.0, "pool": 0.0}
            sink_cost = {"dve": _DVE_COST, "pool": 2100.0}
            ring_free_at = [0.0, 0.0, 0.0]  # exp-finish per 'pp' ring slot
            qkv_q = []  # (('qk',nn,th)|('v',j), pe_ns, closure)
            pv_q = []  # (ready_cursor, pe_ns, closure)
            proj_q = []  # (ready_cursor, pe_ns, closure)
            reserve_q = []  # proj pairs held back for the endgame
            pending = []
            done = set()
            transp_done = {}

            def emit(pe_ns, closure):
                closure()
                st["cursor"] += pe_ns

            def pump_one():
                """Emit one filler item; returns False if nothing available.

                Ready pv first (frees psum + pt rings), then qkv (available
                early, hoardable), then ready proj; as a last resort pop the
                least-unready pv/proj item."""
                if pv_q and pv_q[0][0] <= st["cursor"]:
                    _, pe_ns, cl = pv_q.pop(0)
                    emit(pe_ns, cl)
                    return True
                if qkv_q:
                    _, pe_ns, cl = qkv_q.pop(0)
                    emit(pe_ns, cl)
                    return True
                if proj_q and proj_q[0][0] <= st["cursor"]:
                    _, pe_ns, cl = proj_q.pop(0)
                    emit(pe_ns, cl)
                    return True
                if st["units_left"] == 0 and reserve_q:
                    _, pe_ns, cl = reserve_q.pop(0)
                    emit(pe_ns, cl)
                    return True
                best = None
                for q in (pv_q, proj_q):
                    if q and (best is None or q[0][0] < best[0][0]):
                        best = (q[0], q)
                if best is not None:
                    item, q = best
                    q.remove(item)
                    emit(item[1], item[2])
                    return True
                return False

            def drain_qkv(pred):
                keep = []
                for item in qkv_q:
                    if pred(item[0]):
                        emit(item[1], item[2])
                    else:
                        keep.append(item)
                qkv_q[:] = keep

            def emit_piece(pe_ns, act_ns, sink, closure):
                # keep the count of live exp'd tiles below the pt ring depth
                # (each piece holds one pt tile until its PV consumes it)
                while st["pt_live"] > _PT_LIVE_CAP and pv_q:
                    _, pv_pe, pv_cl = pv_q.pop(0)
                    emit(pv_pe, pv_cl)
                if sink is None:
                    # offloadable full piece: route to DVE fast-exp when Act
                    # would finish it later than DVE and error budget remains
                    t0 = st["cursor"] + _ACT_LAT2
                    fin_act = max(sink_fin["act"], t0) + act_ns
                    fin_dve = max(sink_fin["dve"], t0) + sink_cost["dve"]
                    if (
                        st["offloaded"] < OFFLOAD_CAP
                        and st["units_left"] > _TAIL_GATE
                        and fin_act > fin_dve
                    ):
                        sink = "dve"
                        st["offloaded"] += 1
                    else:
                        sink = "act"
                # ensure the ring slot this piece will reuse has been drained
                # by its exp engine before TensorE reaches the matmuls
                slot = st["ps_idx"] % 3
                st["ps_idx"] += 1
                while ring_free_at[slot] > st["cursor"]:
                    if not pump_one():
                        break
                emit(pe_ns, lambda: closure(sink))
                st["pt_live"] += 1
                cost = act_ns if sink == "act" else sink_cost[sink]
                start = max(sink_fin[sink], st["cursor"] + _ACT_LAT2)
                sink_fin[sink] = start + cost
                if sink == "act":
                    st["act_fin"] = sink_fin["act"]
                # +300: exp drain (init/2) + sem propagation before the psum
                # bank is reusable by TensorE
                ring_free_at[slot] = sink_fin[sink] + _RING_POST

            def flush_one():
                c_, h_, pts_, mts_, act_fin_ = pending.pop(0)
                drain_qkv(lambda k: k[0] == "v" and k[1] <= 4 * c_ + 3)
                nsteps = sum(4 * c_ + ti + 1 for ti in range(4))

                def pv_all():
                    # single item: the shared po-ring slot is held from first
                    # matmul to the norm drain, so no other po user may be
                    # emitted in between (PE is in-order)
                    p_yf = ps.tile([128, 512], F32, tag="po", bufs=2, name="p_yf")
                    p_y = p_yf[:, 0:260]
                    pv_block(p_y, h_, c_, pts_, mts_, (0, 1))
                    pv_block(p_y, h_, c_, pts_, mts_, (2, 3))
                    pv_norm(p_y, h_, c_)
                    st["pt_live"] -= 2 * c_ + 2
                    done.add((c_, h_))
                    if (c_, h_ ^ 1) in done:
                        hp_ = h_ // 2
                        transpose_dma(hp_, c_)
                        tset = transp_done.setdefault(c_, set())
                        tset.add(hp_)
                        if len(tset) == 4:
                            split_ktile(c_, 0)
                            split_ktile(c_, 1)
                            # proj waits for the transpose DMA + hi/lo split
                            # chain (+margin)
                            rdy = st["cursor"] + _PROJ_RDY
                            for cop in range(4):
                                item = (
                                    rdy,
                                    2 * 6 * 512 * 0.5 * PE_CYC,
                                    (lambda c2=c_, cop=cop: proj_pair(c2, cop)),
                                )
                                proj_q.append(item)

                pv_q.append((act_fin_ + 600.0, nsteps * 65 * PE_CYC, pv_all))

            def emit_unit(c, h):
                # data deps: qt col-block h//2 for chunk c up front; kt
                # col-block 4+h//2 drained lazily per piece, weaving the qk
                # fill work between score pieces
                g = h // 2
                drain_qkv(lambda k: k[0] == "qk" and k[1] == g and k[2] == c)
                pts, mts = [], []
                for pe_ns, act_ns, kt_th, sink, piece in att_scores_pieces(
                    h, c, pts, mts
                ):
                    drain_qkv(
                        lambda k: k[0] == "qk"
                        and k[1] == 4 + g
                        and k[2] <= min(kt_th + 1, c)
                    )
                    emit_piece(pe_ns, act_ns, sink, piece)
                pending.append((c, h, pts, mts, st["act_fin"]))
                if len(pending) > _PEND:
                    flush_one()

            # ---- fill qkv queue in data-arrival order ----
            qk_order = [0, 4, 1, 5, 2, 6, 3, 7]
            for th in range(4):
                for nn in qk_order:
                    qkv_q.append(
                        (("qk", nn, th), 1280, lambda nn=nn, th=th: qk_half(nn, th))
                    )
                for j in range(4 * th, 4 * th + 4):
                    qkv_q.append((("v", j), 1280, lambda j=j: v_unit(j)))

            # interleave chunks so the exp-heavy c3/c2 units start as soon as
            # their x data lands and the Act demand is spread evenly; chunk
            # completion staggered (c0 < c1 < c3 < c2) so each chunk's proj
            # fills the next chunk's tail
            unit_order = [
                (0, 0), (0, 1), (0, 2), (0, 3), (0, 4), (0, 5),
                (0, 6), (0, 7), (1, 0), (1, 1), (3, 0), (3, 1),
                (1, 2), (1, 3), (2, 0), (2, 1), (3, 2), (3, 3),
                (1, 4), (1, 5), (2, 2), (2, 3), (3, 4), (3, 5),
                (1, 6), (1, 7), (2, 4), (2, 5), (3, 6), (3, 7),
                (2, 6), (2, 7),
            ]
            assert sorted(unit_order) == sorted(
                (c, h) for c in range(4) for h in range(HL)
            )

            for c, h in unit_order:
                emit_unit(c, h)
                st["units_left"] -= 1
            while pending:
                flush_one()
                for _ in range(4):
                    pump_one()
            proj_q.extend(reserve_q)
            reserve_q[:] = []
            while pv_q or proj_q or qkv_q:
                if not pump_one():
                    break

    nc.compile()
    return nc


# ---------------------------------------------------------------------------
# host side
# ---------------------------------------------------------------------------

_CACHE = {}


def _get_runner():
    if "runner" in _CACHE:
        return _CACHE["runner"]

    import jax
    from jax.experimental.shard_map import shard_map
    from jax.sharding import Mesh, PartitionSpec

    from concourse.bass2jax import (
        _bass_exec_p,
        install_neuronx_cc_hook,
        partition_id_tensor,
    )

    install_neuronx_cc_hook()
    nc = build_nc()
    n_cores = 8

    partition_name = nc.partition_id_tensor.name if nc.partition_id_tensor else None
    in_names = []
    out_names = []
    out_avals = []
    for alloc in nc.m.functions[0].allocations:
        if not isinstance(alloc, mybir.MemoryLocationSet):
            continue
        name = alloc.memorylocations[0].name
        if alloc.kind == "ExternalInput":
            if name != partition_name:
                in_names.append(name)
        elif alloc.kind == "ExternalOutput":
            out_names.append(name)
            out_avals.append(
                jax.core.ShapedArray(tuple(alloc.tensor_shape), mybir.dt.np(alloc.dtype))
            )
    n_params = len(in_names)
    all_names = in_names + out_names
    if partition_name is not None:
        all_names = all_names + [partition_name]

    def _body(*args):
        operands = list(args)
        if partition_name is not None:
            operands.append(partition_id_tensor())
        outs = _bass_exec_p.bind(
            *operands,
            out_avals=tuple(out_avals),
            in_names=tuple(all_names),
            out_names=tuple(out_names),
            lowering_input_output_aliases=(),
            sim_require_finite=True,
            sim_require_nnan=True,
            nc=nc,
        )
        return tuple(outs)

    devices = jax.devices()[:n_cores]
    mesh = Mesh(np.asarray(devices), ("core",))
    n_outs = len(out_names)
    fn = jax.jit(
        shard_map(
            _body,
            mesh=mesh,
            in_specs=(PartitionSpec("core"),) * (n_params + n_outs),
            out_specs=(PartitionSpec("core"),) * n_outs,
            check_rep=False,
        ),
        keep_unused=True,
    )

    runner = {
        "fn": fn,
        "in_names": in_names,
        "out_names": out_names,
        "out_avals": out_avals,
        "n_cores": n_cores,
        "jax": jax,
    }
    _CACHE["runner"] = runner
    return runner


def _pack_pairs(a):
    """[C, N] -> [128, NK2, 2, N]: row 256*k + 128*s + p -> [p, k, s]."""
    n = a.shape[1]
    return np.ascontiguousarray(
        a.reshape(NK2, 2, 128, n).transpose(2, 0, 1, 3)
    )


def _hi_lo(a):
    from ml_dtypes import float8_e4m3

    hi = a.astype(float8_e4m3)
    lo = (a - hi.astype(np.float32)).astype(float8_e4m3)
    return hi, lo


def _prepare_in_maps(x, w_attn, b_attn, w_proj, b_proj):
    x = np.asarray(x, dtype=np.float32)
    w_attn = np.asarray(w_attn, dtype=np.float32)
    b_attn = np.asarray(b_attn, dtype=np.float32)
    w_proj = np.asarray(w_proj, dtype=np.float32)
    b_proj = np.asarray(b_proj, dtype=np.float32)

    in_maps = []
    for core in range(8):
        b = core // 2
        c0 = CL * (core % 2)

        xT = np.ascontiguousarray(x[b].T) * FP8_SCALE_X  # [C, T] f32
        x_hi, x_lo = _hi_lo(xT)
        xp_h = _pack_pairs(x_hi)  # [128, 4, 2, T]
        xp_l = _pack_pairs(x_lo)

        w_all = np.concatenate(
            [
                w_attn[:, c0 : c0 + CL] * 0.125,
                w_attn[:, C + c0 : C + c0 + CL],
                w_attn[:, 2 * C + c0 : 2 * C + c0 + CL],
            ],
            axis=1,
        )  # [C, 1536] = q(512) | k(512) | v(512)
        w_all = w_all * FP8_SCALE_W
        w_hi, w_lo = _hi_lo(w_all)
        wp_h = _pack_pairs(w_hi)  # [128, 4, 2, 1536]
        wp_l = _pack_pairs(w_lo)

        def pack_first(wp, xp):
            # w cols nn0(q0:128), nn1(q128:256), nn4(k512:640), nn5(k640:768)
            # then x t0:512
            return np.ascontiguousarray(
                np.concatenate(
                    [
                        wp[:, :, :, 0:128],
                        wp[:, :, :, 128:256],
                        wp[:, :, :, 512:640],
                        wp[:, :, :, 640:768],
                        xp[:, :, :, 0:512],
                    ],
                    axis=3,
                )
            )

        def pack_wrest(wp):
            # nn2(q256:384), nn3(q384:512), nn6(k768:896), nn7(k896:1024), v
            return np.ascontiguousarray(
                np.concatenate(
                    [
                        wp[:, :, :, 256:384],
                        wp[:, :, :, 384:512],
                        wp[:, :, :, 768:896],
                        wp[:, :, :, 896:1024],
                        wp[:, :, :, 1024:1536],
                    ],
                    axis=3,
                )
            )

        first_hh = pack_first(wp_h, xp_h)
        first_ll = pack_first(wp_l, xp_l)
        wrest_hh = pack_wrest(wp_h)
        wrest_ll = pack_wrest(wp_l)
        xrest_hh = np.ascontiguousarray(xp_h[:, :, :, 512:T])
        xrest_ll = np.ascontiguousarray(xp_l[:, :, :, 512:T])

        # w_proj fp8 hi/lo, DR-packed: row 256k+128s+p -> [p, k, s, :]
        wpj = w_proj[c0 : c0 + CL, :] * FP8_SCALE_W
        wpj_hi, wpj_lo = _hi_lo(wpj)
        wpjhh = np.ascontiguousarray(
            wpj_hi.reshape(2, 2, 128, C).transpose(2, 0, 1, 3)
        )
        wpjll = np.ascontiguousarray(
            wpj_lo.reshape(2, 2, 128, C).transpose(2, 0, 1, 3)
        )

        b_q = b_attn[c0 : c0 + CL] * 0.125
        b_k = b_attn[C + c0 : C + c0 + CL]
        bqk = np.ascontiguousarray(
            np.concatenate([b_q, b_k]).reshape(8, 128).T.astype(np.float32)
        )
        b_v = b_attn[2 * C + c0 : 2 * C + c0 + CL].astype(np.float32)
        bv = np.ascontiguousarray(np.broadcast_to(b_v[None, :], (128, CL)))
        if core % 2 == 0:
            bp = np.ascontiguousarray(b_proj.reshape(8, 128).T.astype(np.float32))
        else:
            bp = np.zeros((128, 8), dtype=np.float32)

        in_maps.append(
            {
                "first_hh": first_hh,
                "first_ll": first_ll,
                "wrest_hh": wrest_hh,
                "wrest_ll": wrest_ll,
                "xrest_hh": xrest_hh,
                "xrest_ll": xrest_ll,
                "wpjhh": wpjhh,
                "wpjll": wpjll,
                "bqk": bqk,
                "bv": bv,
                "bproj": bp,
            }
        )
    return in_maps


def _run_device(in_maps):
    r = _get_runner()
    jax = r["jax"]
    n = r["n_cores"]
    per_core = [[np.asarray(m[name]) for name in r["in_names"]] for m in in_maps]
    concat_in = [
        np.concatenate([per_core[c][i] for c in range(n)], axis=0)
        for i in range(len(r["in_names"]))
    ]
    concat_zero = [
        np.zeros((n * a.shape[0], *a.shape[1:]), a.dtype) for a in r["out_avals"]
    ]
    outs = r["fn"](*[jax.device_put(a) for a in concat_in + concat_zero])
    jax.block_until_ready(outs)
    (outT,) = [np.asarray(o) for o in outs]
    return outT.reshape(n, C, T)


def kernel(x, w_attn, b_attn, w_proj, b_proj):
    in_maps = _prepare_in_maps(x, w_attn, b_attn, w_proj, b_proj)
    outT = _run_device(in_maps)
    out = np.empty((B, T, C), dtype=np.float32)
    for b in range(B):
        out[b] = (outT[2 * b].astype(np.float32) + outT[2 * b + 1].astype(np.float32)).T
    return out


# revision 80
# speedup vs baseline: 1.2605x; 1.0121x over previous
"""Causal self-attention (B=4, T=2048, C=1024, H=16) on 8 TRN2 NeuronCores.

Sharding: data-parallel over B (4) x tensor-parallel over heads (2 halves of 8
heads). Core c handles batch c//2, heads 8*(c%2) .. 8*(c%2)+8. Each core runs
the full pipeline for its (batch, head-half); the host sums core pairs and
transposes.

Structure (chosen to minimise TensorE row-streaming cost):
- QKV projection in fp8(e4m3) hi/lo error-compensated DoubleRow matmuls:
  x = x_hi + x_lo, w = w_hi + w_lo; x@w ~ x_hi w_hi + x_hi w_lo + x_lo w_hi.
  DoubleRow packs two 128-row k-tiles per instruction.
- Scores S^T[k,q] in fp16 with block-causal skipping (128-row x 128-col
  granularity): only lower-triangular blocks are computed/exp'd.
- Exp on the Act engine; diagonal 128x128 triangles masked on GPSIMD.
- PV with the probability block as the *stationary* operand: out[q,65] per
  128-q-block (65th vaug column of ones gives softmax denominators), so each
  accumulation step streams only 65 rows.
- Per-q normalisation via DVE reciprocal + tensor_scalar broadcast.
- y[q,d] head pairs transposed via the DMA XBAR (dma_start_transpose), not
  the PE, freeing both TensorE cycles and a PSUM bank.
- Startup: inputs land via a few large partition-major DMAs (first/wrest/
  xrest); warm-up matmuls on a zero tile keep TensorE busy (and its clock
  ramping) until the first real operands arrive.
- A pacing scheduler co-simulates the Act engine's exp backlog and weaves
  qkv/pv/proj work between score pieces so TensorE never waits on exp.
- Output stored fp16; host sums head-half pairs in f32.
"""

import sys

if "/opt/trn_rl_repo" not in sys.path:
    sys.path.insert(0, "/opt/trn_rl_repo")

from contextlib import ExitStack

import numpy as np

import concourse.tile as tile
from concourse import bacc, mybir

F32 = mybir.dt.float32
FP16 = mybir.dt.float16
FP8 = mybir.dt.float8e4
DR = mybir.MatmulPerfMode.DoubleRow
EXP = mybir.ActivationFunctionType.Exp

B, T, C, H = 4, 2048, 1024, 16
HL = 8  # heads per core
HD = 64  # head dim
CL = HL * HD  # local width (512)
W3 = 3 * CL  # qkv local col count (1536)
NK2 = 4  # fp8 k-tile pairs over C (256 each)
NTT = T // 128  # 16 t-blocks of 128
# fp8 range fix: lo-parts of x (~2%) and w (~0.08%) underflow e4m3 subnormals
# (min 2^-9); scale operands up on the host, descale in the PSUM->SBUF copy.
FP8_SCALE_X = 8.0
FP8_SCALE_W = 64.0
FP8_DESCALE = 1.0 / (FP8_SCALE_X * FP8_SCALE_W)

# cost-model constants (TimelineSim) used by the pacing scheduler
PE_CYC = 1.0 / 2.4  # ns per cycle, warm
ACT_CYC = 1.0 / 1.2
ACT_BUBBLE = 370.0  # per-instruction SBUF access bubble on Act
ACT_LAT = 420.0  # psum-ready -> act-start latency (pipeline+sem)
N_WARM = 14  # warm-up matmuls before the first DMA lands

# fast-exp (Schraudolph, fp16 bit trick): exp(s) ~ bitcast_fp16(int16(s*FE_A
# + FE_B)). Sawtooth rel err ~1.8% rms, mean ratio 1.0 (calibrated); used on
# a minority of off-diagonal score pieces to offload exp work from the Act
# engine to DVE/GPSIMD. Softmax normalisation absorbs the common-mode part.
FE_A = float(np.log2(np.e) * 1024.0)  # 1477.3197
FE_B = 15301.0
I16 = mybir.dt.int16
OFFLOAD_CAP = 40  # max fast-exp'd full pieces (error budget)

import os as _os
N_WARM = int(_os.environ.get("K_NWARM", N_WARM))
OFFLOAD_CAP = int(_os.environ.get("K_OFFCAP", 48))
_PT_LIVE_CAP = int(_os.environ.get("K_PTLIVE", 16))
_DVE_COST = float(_os.environ.get("K_DVECOST", 2000.0))
_ACT_LAT2 = float(_os.environ.get("K_ACTLAT", 420.0))
_RING_POST = float(_os.environ.get("K_RINGPOST", 650.0))
_PROJ_RDY = float(_os.environ.get("K_PROJRDY", 9000.0))
_TAIL_GATE = int(_os.environ.get("K_TAILGATE", 0))
_PEND = int(_os.environ.get("K_PEND", 2))
_OFF_BIAS = float(_os.environ.get("K_OFFBIAS", 0.0))


def build_nc():
    nc = bacc.Bacc(None)

    # DRAM layouts, partition-major. first: w q-cols nn0,nn1 + k-cols nn4,nn5
    # + x t0:512. wrest: w q nn2,nn3 + k nn6,nn7 + v cols. xrest: x t512:2048.
    first_hh_d = nc.declare_dram_parameter("first_hh", [128, NK2, 2, 1024], FP8, isOutput=False)
    first_ll_d = nc.declare_dram_parameter("first_ll", [128, NK2, 2, 1024], FP8, isOutput=False)
    wrest_hh_d = nc.declare_dram_parameter("wrest_hh", [128, NK2, 2, 1024], FP8, isOutput=False)
    wrest_ll_d = nc.declare_dram_parameter("wrest_ll", [128, NK2, 2, 1024], FP8, isOutput=False)
    xrest_hh_d = nc.declare_dram_parameter("xrest_hh", [128, NK2, 2, 1536], FP8, isOutput=False)
    xrest_ll_d = nc.declare_dram_parameter("xrest_ll", [128, NK2, 2, 1536], FP8, isOutput=False)
    # w_proj fp8 hi/lo, DoubleRow-packed over the 512 contract rows:
    # [p, ktile, s, co] with contract row = 256*ktile + 128*s + p
    wpjhh_d = nc.declare_dram_parameter("wpjhh", [128, 2, 2, C], FP8, isOutput=False)
    wpjll_d = nc.declare_dram_parameter("wpjll", [128, 2, 2, C], FP8, isOutput=False)
    bqk_d = nc.declare_dram_parameter("bqk", [128, 8], F32, isOutput=False)
    bv_d = nc.declare_dram_parameter("bv", [128, CL], F32, isOutput=False)
    bproj_d = nc.declare_dram_parameter("bproj", [128, 8], F32, isOutput=False)
    outT_d = nc.declare_dram_parameter("outT", [8, 128, T], FP16, isOutput=True)

    with tile.TileContext(nc) as tc, ExitStack() as ctx:
        persist = ctx.enter_context(tc.tile_pool(name="persist", bufs=1))
        first_hh = persist.tile([128, NK2, 2, 1024], FP8, tag="fhh")
        first_ll = persist.tile([128, NK2, 2, 1024], FP8, tag="fll")
        wrest_hh = persist.tile([128, NK2, 2, 1024], FP8, tag="wrhh")
        wrest_ll = persist.tile([128, NK2, 2, 1024], FP8, tag="wrll")
        xrest_hh = persist.tile([128, NK2, 2, 1536], FP8, tag="xrhh")
        xrest_ll = persist.tile([128, NK2, 2, 1536], FP8, tag="xrll")
        # q^T / k^T blocks: nn 0..3 = q cols, 4..7 = k cols; [col128, T]
        qkT = [persist.tile([128, T], FP16, tag=f"qkT{nn}", name=f"qkT{nn}") for nn in range(8)]
        # v (+ ones col) per 128-t-block: [t128, head, 65]
        vaug = [persist.tile([128, HL, 65], FP16, tag=f"vaug{j}", name=f"vaug{j}") for j in range(NTT)]
        # y head-pairs [q128, qblock, dpair]; transposed chunks live in
        # per-chunk ring tiles (see transpose_dma / split_chunk below)
        y_pair = [persist.tile([128, NTT, 128], FP16, tag=f"yp{hp}", name=f"yp{hp}") for hp in range(4)]
        wpjhh_sb = persist.tile([128, 2, 2, C], FP8, tag="wpjhh")
        wpjll_sb = persist.tile([128, 2, 2, C], FP8, tag="wpjll")
        bqk_sb = persist.tile([128, 8], F32, tag="bqk")
        bv_sb = persist.tile([128, CL], F32, tag="bv")
        bproj_sb = persist.tile([128, 8], F32, tag="bproj")
        warm = persist.tile([128, 512], FP16, tag="warm")

        def w_slice(nn, k):
            """Stationary w tile for qk col-block nn, fp8 pair k: (hh, ll)."""
            src_h, src_l, off = {
                0: (first_hh, first_ll, 0),
                1: (first_hh, first_ll, 128),
                4: (first_hh, first_ll, 256),
                5: (first_hh, first_ll, 384),
                2: (wrest_hh, wrest_ll, 0),
                3: (wrest_hh, wrest_ll, 128),
                6: (wrest_hh, wrest_ll, 256),
                7: (wrest_hh, wrest_ll, 384),
            }[nn]
            return (src_h[:, k, :, off : off + 128], src_l[:, k, :, off : off + 128])

        def x_span(k, t0, t1):
            """Moving x tile [128, 2, t1-t0] for fp8 pair k: (hh, ll)."""
            if t1 <= 512:
                return (
                    first_hh[:, k, :, 512 + t0 : 512 + t1],
                    first_ll[:, k, :, 512 + t0 : 512 + t1],
                )
            assert t0 >= 512
            return (
                xrest_hh[:, k, :, t0 - 512 : t1 - 512],
                xrest_ll[:, k, :, t0 - 512 : t1 - 512],
            )

        def wv_slice(k):
            return (wrest_hh[:, k, :, 512:1024], wrest_ll[:, k, :, 512:1024])

        # ---- warm-up + input DMAs ----
        nc.vector.memset(warm[:], 0.0)

        with (
            tc.tile_pool(name="work", bufs=1) as work,
            tc.tile_pool(name="ps", bufs=1, space="PSUM") as ps,
        ):
            # warm-up: one accumulation group — same-engine ordering only, so
            # the matmuls run back-to-back and ramp the PE clock while the
            # first input DMAs are in flight
            p_w = ps.tile([128, 512], F32, tag="po", bufs=2)
            for i in range(N_WARM):
                nc.tensor.matmul(
                    p_w[:], warm[:, 0:128], warm[:],
                    start=(i == 0), stop=(i == N_WARM - 1),
                )

            # input DMAs: big tensors alternate the SP/Act HWDGE queues in
            # landing order first (w qk nn0/1/4/5 + x chunk0), wrest (w qk
            # rest + v), x rest; small/late tensors go via the parallel
            # gpsimd SWDGE path so they don't hold up the HWDGE pipeline.
            nc.gpsimd.dma_start(bqk_sb[:], bqk_d[:])
            nc.gpsimd.dma_start(bv_sb[:], bv_d[:])
            nc.sync.dma_start(first_hh[:], first_hh_d[:])
            nc.scalar.dma_start(first_ll[:], first_ll_d[:])
            nc.sync.dma_start(wrest_hh[:], wrest_hh_d[:])
            nc.scalar.dma_start(wrest_ll[:], wrest_ll_d[:])
            s1 = slice(0, 512)
            nc.sync.dma_start(xrest_hh[:, :, :, s1], xrest_hh_d[:, :, :, s1])
            nc.scalar.dma_start(xrest_ll[:, :, :, s1], xrest_ll_d[:, :, :, s1])
            for cc in range(1, 3):
                s = slice(cc * 512, cc * 512 + 512)
                nc.sync.dma_start(xrest_hh[:, :, :, s], xrest_hh_d[:, :, :, s])
                nc.scalar.dma_start(xrest_ll[:, :, :, s], xrest_ll_d[:, :, :, s])
            nc.gpsimd.dma_start(wpjhh_sb[:], wpjhh_d[:])
            nc.gpsimd.dma_start(wpjll_sb[:], wpjll_d[:])
            nc.gpsimd.dma_start(bproj_sb[:], bproj_d[:])
            for j in range(NTT):
                nc.vector.memset(vaug[j][:, :, 64], 1.0)

            def pt_tile():
                return work.tile([128, 1024], FP16, tag="pt", bufs=22, name="pt")

            def fp8_group(psum_region, stats, movs):
                """12 DoubleRow matmuls: hi*hi + hi*lo + lo*hi over 8 k-tiles.

                stats/movs: lists over k of (hh, ll) AP pairs."""
                n = 0
                for si, mi in ((0, 0), (0, 1), (1, 0)):
                    for k in range(NK2):
                        nc.tensor.matmul(
                            psum_region,
                            stats[k][si],
                            movs[k][mi],
                            start=(n == 0),
                            stop=(n == 3 * NK2 - 1),
                            perf_mode=DR,
                        )
                        n += 1

            def v_unit(j):
                """v projection for t-block j -> vaug[j] (+bias)."""
                p_v = ps.tile([128, 512], F32, tag="po", bufs=2)
                fp8_group(
                    p_v[:],
                    [x_span(k, j * 128, (j + 1) * 128) for k in range(NK2)],
                    [wv_slice(k) for k in range(NK2)],
                )
                nc.vector.scalar_tensor_tensor(
                    vaug[j][:, :, 0:64],
                    p_v[:].rearrange("p (h c) -> p h c", h=HL),
                    FP8_DESCALE,
                    bv_sb[:].rearrange("p (h c) -> p h c", h=HL),
                    mybir.AluOpType.mult,
                    mybir.AluOpType.add,
                )

            def qk_half(nn, th):
                """q^T/k^T col-block nn for t-half th (512 wide) -> qkT[nn]."""
                p_qk = ps.tile([128, 512], F32, tag="po", bufs=2)
                ts0 = th * 512
                fp8_group(
                    p_qk[:],
                    [w_slice(nn, k) for k in range(NK2)],
                    [x_span(k, ts0, ts0 + 512) for k in range(NK2)],
                )
                nc.vector.tensor_scalar(
                    qkT[nn][:, ts0 : ts0 + 512],
                    p_qk[:],
                    FP8_DESCALE,
                    bqk_sb[:, nn : nn + 1],
                    mybir.AluOpType.mult,
                    mybir.AluOpType.add,
                )

            def att_scores_pieces(h, c, pts, mts):
                """Score pieces for (h, c): each closure emits one psum's
                matmuls + exp; the last also emits the 4 diagonal masks.
                Returns list of (pe_ns, act_ns, closure)."""
                poff = (h % 2) * 64
                kt = qkT[4 + h // 2]
                qt = qkT[h // 2]
                qs = slice(c * 512, (c + 1) * 512)
                pieces = []

                def full_pair(jp, sink):
                    p_s = ps.tile([128, 1024], F32, tag="pp", bufs=3)
                    for half in range(2):
                        j = 2 * jp + half
                        nc.tensor.matmul(
                            p_s[:, half * 512 : half * 512 + 512],
                            kt[poff : poff + 64, j * 128 : (j + 1) * 128],
                            qt[poff : poff + 64, qs],
                            start=True,
                            stop=True,
                        )
                    pt = pt_tile()
                    if sink == "act":
                        nc.scalar.activation(pt[:], p_s[:], EXP)
                    else:
                        eng = nc.vector if sink == "dve" else nc.gpsimd
                        eng.tensor_scalar(
                            pt[:].bitcast(I16),
                            p_s[:],
                            FE_A,
                            FE_B,
                            mybir.AluOpType.mult,
                            mybir.AluOpType.add,
                        )
                    pts.append(pt)

                def partial(pp_i):
                    p_s = ps.tile([128, 1024], F32, tag="pp", bufs=3)
                    off = 0
                    for half in range(2):
                        ti = 2 * pp_i + half
                        w = 512 - 128 * ti
                        j = 4 * c + ti
                        nc.tensor.matmul(
                            p_s[:, off : off + w],
                            kt[poff : poff + 64, j * 128 : (j + 1) * 128],
                            qt[poff : poff + 64, c * 512 + 128 * ti : (c + 1) * 512],
                            start=True,
                            stop=True,
                        )
                        off += w
                    pt = pt_tile()
                    nc.scalar.activation(pt[:, 0:off], p_s[:, 0:off], EXP)
                    pts.append(pt)

                def masks():
                    # diag triangles of (ti0,ti1) sit at offsets 0/512 of the
                    # first partial tile, (ti2,ti3) at 0/256 of the second:
                    # batch each pair as one strided affine_select
                    for pp_i, astr in ((0, 512), (1, 256)):
                        pt = pts[2 * c + pp_i]
                        src = pt[:, 0 : 2 * astr].rearrange(
                            "p (a w) -> p a w", a=2
                        )[:, :, 0:128]
                        mt = work.tile(
                            [128, 2, 128], FP16, tag="mt", bufs=8, name="mt"
                        )
                        nc.gpsimd.affine_select(
                            mt[:],
                            src,
                            pattern=[[0, 2], [1, 128]],
                            compare_op=mybir.AluOpType.is_ge,
                            fill=0.0,
                            base=0,
                            channel_multiplier=-1,
                        )
                        mts.append(mt)

                # full pieces may be routed to DVE fast-exp at emission time
                # (sink=None = scheduler's choice, capped by error budget)
                for jp in range(2 * c):
                    pieces.append(
                        (427, 1100, jp // 2, None,
                         (lambda sink, jp=jp: full_pair(jp, sink)))
                    )
                pieces.append((373, 995, c, "act", lambda sink: partial(0)))

                def last(sink):
                    partial(1)
                    masks()

                pieces.append((160, 568, c, "act", last))
                return pieces

            def pv_block(p_y, h, c, pts, mts, tis):
                for ti in tis:
                    i = 4 * c + ti
                    ys = slice(ti * 65, ti * 65 + 65)
                    for j in range(i + 1):
                        if j == i:
                            blk = mts[ti // 2][:, ti % 2, :]
                        elif j >= 4 * c:
                            tj = j - 4 * c
                            off = (
                                0 if tj % 2 == 0 else 512 - 128 * (tj - 1)
                            ) + 128 * (ti - tj)
                            blk = pts[2 * c + tj // 2][:, off : off + 128]
                        else:
                            blk = pts[j // 2][
                                :,
                                (j % 2) * 512 + 128 * ti : (j % 2) * 512
                                + 128 * ti
                                + 128,
                            ]
                        nc.tensor.matmul(
                            p_y[:, ys],
                            blk,
                            vaug[j][:, h, :],
                            start=(j == 0),
                            stop=(j == i),
                        )

            def pv_norm(p_y, h, c):
                hp, doff = h // 2, (h % 2) * 64
                r = work.tile([128, 4], F32, tag="r", bufs=8, name="r")
                nc.vector.reciprocal(r[:], p_y[:, 64::65])
                nc.vector.tensor_tensor(
                    y_pair[hp][:, 4 * c : 4 * c + 4, doff : doff + 64],
                    p_y[:].rearrange("p (t k) -> p t k", t=4)[:, :, 0:64],
                    r[:].rearrange("p t -> p t ()").to_broadcast([128, 4, 64]),
                    mybir.AluOpType.mult,
                )

            # per-chunk transposed-y ring tiles: ytp fp16 (transpose target,
            # split source) and fp8 hi/lo (DoubleRow proj moving operands)
            ytp_t = {}  # (c, hp) -> fp16 [128, 4, 128]
            yhl_t = {}  # (c, k) -> (hi [128, 2, 512], lo [128, 2, 512])

            def transpose_dma(hp, c):
                """Block-transpose y_pair[hp] chunk c via the DMA XBAR:
                in [q=128,(qb,d)=512] -> out [d=128][qb=4][q=128]."""
                yt = work.tile(
                    [128, 4, 128], FP16, tag=f"ytp{hp}", bufs=2, name="yt"
                )
                ytp_t[(c, hp)] = yt
                nc.sync.dma_start_transpose(yt[:], y_pair[hp][:, 4 * c : 4 * c + 4, :])

            def split_ktile(c, k):
                """fp16 y^T chunk (head-pair ktile k) -> fp8 hi/lo (scaled by
                FP8_SCALE_X) for the DoubleRow projection."""
                if True:
                    yhi = work.tile(
                        [128, 2, 512], FP8, tag=f"yhi{k}", bufs=2, name="yhi"
                    )
                    ylo = work.tile(
                        [128, 2, 512], FP8, tag=f"ylo{k}", bufs=2, name="ylo"
                    )
                    yhl_t[(c, k)] = (yhi, ylo)
                    for s in range(2):
                        src = ytp_t[(c, 2 * k + s)][:].rearrange("p a w -> p (a w)")
                        nc.vector.tensor_scalar(
                            yhi[:, s, :],
                            src,
                            FP8_SCALE_X,
                            0.0,
                            mybir.AluOpType.mult,
                            mybir.AluOpType.add,
                        )
                        nc.vector.scalar_tensor_tensor(
                            ylo[:, s, :],
                            src,
                            FP8_SCALE_X,
                            yhi[:, s, :],
                            mybir.AluOpType.mult,
                            mybir.AluOpType.subtract,
                        )

            def proj_pair(c, cop):
                """Output projection for co pair (2*cop, 2*cop+1) of chunk c:
                fp8 DoubleRow hi/lo (3 passes x 2 ktiles), drained to a
                staging tile then DMA'd per co."""
                o_s = work.tile([128, 2, 512], FP16, tag="os", bufs=4, name="os")
                for half in range(2):
                    co = 2 * cop + half
                    cs = slice(co * 128, (co + 1) * 128)
                    p_o = ps.tile([128, 512], F32, tag="po", bufs=2)
                    n = 0
                    # y_hi-only passes first: the first four matmuls can start
                    # before the (later) y_lo split lands
                    for wi, yi in ((0, 0), (1, 0), (0, 1)):
                        for k in range(2):
                            w_ap = (wpjhh_sb if wi == 0 else wpjll_sb)[:, k, :, cs]
                            y_ap = yhl_t[(c, k)][yi][:]
                            nc.tensor.matmul(
                                p_o[:],
                                w_ap,
                                y_ap,
                                start=(n == 0),
                                stop=(n == 5),
                                perf_mode=DR,
                            )
                            n += 1
                    # psum drains: DVE, except the last chunk's odd halves
                    # which go to the by-then-idle Act engine (GPSIMD has no
                    # PSUM access)
                    nc.vector.tensor_scalar(
                        o_s[:, half, :],
                        p_o[:],
                        FP8_DESCALE,
                        bproj_sb[:, co : co + 1],
                        mybir.AluOpType.mult,
                        mybir.AluOpType.add,
                    )
                    # per-half DMA: each fires as soon as its drain is done
                    # (SP queue only — a sem-waiting DMA on the Act queue
                    # would block exp)
                    nc.sync.dma_start(
                        outT_d[co, :, c * 512 : (c + 1) * 512],
                        o_s[:, half, :],
                    )

            # ---------- pacing scheduler ----------
            # co-simulates Act's exp backlog (cost-model constants) and pumps
            # qkv/pv/proj filler between score pieces so the 3-deep score
            # psum ring never blocks TensorE on the Act engine.
            st = {"cursor": 0.0, "act_fin": 0.0, "ps_idx": 0, "pt_live": 0,
                  "offloaded": 0, "units_left": 32}
            # per-sink backlog model; dve/pool per-piece costs inflated for
            # their unmodelled other work (drains / masks)
            sink_fin = {"act": 0.0, "dve": 0.0, "pool": 0.0}
            sink_cost = {"dve": _DVE_COST, "pool": 2100.0}
            ring_free_at = [0.0, 0.0, 0.0]  # exp-finish per 'pp' ring slot
            qkv_q = []  # (('qk',nn,th)|('v',j), pe_ns, closure)
            pv_q = []  # (ready_cursor, pe_ns, closure)
            proj_q = []  # (ready_cursor, pe_ns, closure)
            reserve_q = []  # proj pairs held back for the endgame
            pending = []
            done = set()
            transp_done = {}

            def emit(pe_ns, closure):
                closure()
                st["cursor"] += pe_ns

            def pump_one():
                """Emit one filler item; returns False if nothing available.

                Ready pv first (frees psum + pt rings), then qkv (available
                early, hoardable), then ready proj; as a last resort pop the
                least-unready pv/proj item."""
                if pv_q and pv_q[0][0] <= st["cursor"]:
                    _, pe_ns, cl = pv_q.pop(0)
                    emit(pe_ns, cl)
                    return True
                if qkv_q:
                    _, pe_ns, cl = qkv_q.pop(0)
                    emit(pe_ns, cl)
                    return True
                if proj_q and proj_q[0][0] <= st["cursor"]:
                    _, pe_ns, cl = proj_q.pop(0)
                    emit(pe_ns, cl)
                    return True
                if st["units_left"] == 0 and reserve_q:
                    _, pe_ns, cl = reserve_q.pop(0)
                    emit(pe_ns, cl)
                    return True
                best = None
                for q in (pv_q, proj_q):
                    if q and (best is None or q[0][0] < best[0][0]):
                        best = (q[0], q)
                if best is not None:
                    item, q = best
                    q.remove(item)
                    emit(item[1], item[2])
                    return True
                return False

            def drain_qkv(pred):
                keep = []
                for item in qkv_q:
                    if pred(item[0]):
                        emit(item[1], item[2])
                    else:
                        keep.append(item)
                qkv_q[:] = keep

            def emit_piece(pe_ns, act_ns, sink, closure):
                # keep the count of live exp'd tiles below the pt ring depth
                # (each piece holds one pt tile until its PV consumes it)
                while st["pt_live"] > _PT_LIVE_CAP and pv_q:
                    _, pv_pe, pv_cl = pv_q.pop(0)
                    emit(pv_pe, pv_cl)
                if sink is None:
                    # offloadable full piece: route to DVE fast-exp when Act
                    # would finish it later than DVE and error budget remains
                    t0 = st["cursor"] + _ACT_LAT2
                    fin_act = max(sink_fin["act"], t0) + act_ns
                    fin_dve = max(sink_fin["dve"], t0) + sink_cost["dve"]
                    if (
                        st["offloaded"] < OFFLOAD_CAP
                        and st["units_left"] > _TAIL_GATE
                        and fin_act > fin_dve
                    ):
                        sink = "dve"
                        st["offloaded"] += 1
                    else:
                        sink = "act"
                # ensure the ring slot this piece will reuse has been drained
                # by its exp engine before TensorE reaches the matmuls
                slot = st["ps_idx"] % 3
                st["ps_idx"] += 1
                while ring_free_at[slot] > st["cursor"]:
                    if not pump_one():
                        break
                emit(pe_ns, lambda: closure(sink))
                st["pt_live"] += 1
                cost = act_ns if sink == "act" else sink_cost[sink]
                start = max(sink_fin[sink], st["cursor"] + _ACT_LAT2)
                sink_fin[sink] = start + cost
                if sink == "act":
                    st["act_fin"] = sink_fin["act"]
                # +300: exp drain (init/2) + sem propagation before the psum
                # bank is reusable by TensorE
                ring_free_at[slot] = sink_fin[sink] + _RING_POST

            def flush_one():
                c_, h_, pts_, mts_, act_fin_ = pending.pop(0)
                drain_qkv(lambda k: k[0] == "v" and k[1] <= 4 * c_ + 3)
                nsteps = sum(4 * c_ + ti + 1 for ti in range(4))

                def pv_all():
                    # single item: the shared po-ring slot is held from first
                    # matmul to the norm drain, so no other po user may be
                    # emitted in between (PE is in-order)
                    p_yf = ps.tile([128, 512], F32, tag="po", bufs=2, name="p_yf")
                    p_y = p_yf[:, 0:260]
                    pv_block(p_y, h_, c_, pts_, mts_, (0, 1))
                    pv_block(p_y, h_, c_, pts_, mts_, (2, 3))
                    pv_norm(p_y, h_, c_)
                    st["pt_live"] -= 2 * c_ + 2
                    done.add((c_, h_))
                    if (c_, h_ ^ 1) in done:
                        hp_ = h_ // 2
                        transpose_dma(hp_, c_)
                        tset = transp_done.setdefault(c_, set())
                        tset.add(hp_)
                        if len(tset) == 4:
                            split_ktile(c_, 0)
                            split_ktile(c_, 1)
                            # proj waits for the transpose DMA + hi/lo split
                            # chain (+margin)
                            rdy = st["cursor"] + _PROJ_RDY
                            for cop in range(4):
                                item = (
                                    rdy,
                                    2 * 6 * 512 * 0.5 * PE_CYC,
                                    (lambda c2=c_, cop=cop: proj_pair(c2, cop)),
                                )
                                proj_q.append(item)

                pv_q.append((act_fin_ + 600.0, nsteps * 65 * PE_CYC, pv_all))

            def emit_unit(c, h):
                # data deps: qt col-block h//2 for chunk c up front; kt
                # col-block 4+h//2 drained lazily per piece, weaving the qk
                # fill work between score pieces
                g = h // 2
                drain_qkv(lambda k: k[0] == "qk" and k[1] == g and k[2] == c)
                pts, mts = [], []
                for pe_ns, act_ns, kt_th, sink, piece in att_scores_pieces(
                    h, c, pts, mts
                ):
                    drain_qkv(
                        lambda k: k[0] == "qk"
                        and k[1] == 4 + g
                        and k[2] <= min(kt_th + 1, c)
                    )
                    emit_piece(pe_ns, act_ns, sink, piece)
                pending.append((c, h, pts, mts, st["act_fin"]))
                if len(pending) > _PEND:
                    flush_one()

            # ---- fill qkv queue in data-arrival order ----
            qk_order = [0, 4, 1, 5, 2, 6, 3, 7]
            for th in range(4):
                for nn in qk_order:
                    qkv_q.append(
                        (("qk", nn, th), 1280, lambda nn=nn, th=th: qk_half(nn, th))
                    )
                for j in range(4 * th, 4 * th + 4):
                    qkv_q.append((("v", j), 1280, lambda j=j: v_unit(j)))

            # interleave chunks so the exp-heavy c3/c2 units start as soon as
            # their x data lands and the Act demand is spread evenly; chunk
            # completion staggered (c0 < c1 < c3 < c2) so each chunk's proj
            # fills the next chunk's tail
            unit_order = [
                (0, 0), (0, 1), (0, 2), (0, 3), (0, 4), (0, 5),
                (0, 6), (0, 7), (1, 0), (1, 1), (3, 0), (3, 1),
                (1, 2), (1, 3), (2, 0), (2, 1), (3, 2), (3, 3),
                (1, 4), (1, 5), (2, 2), (2, 3), (3, 4), (3, 5),
                (1, 6), (1, 7), (2, 4), (2, 5), (3, 6), (3, 7),
                (2, 6), (2, 7),
            ]
            assert sorted(unit_order) == sorted(
                (c, h) for c in range(4) for h in range(HL)
            )

            for c, h in unit_order:
                emit_unit(c, h)
                st["units_left"] -= 1
            _EPI = int(_os.environ.get("K_EPI", 2))
            while pending:
                flush_one()
                for _ in range(_EPI):
                    pump_one()
            proj_q.extend(reserve_q)
            reserve_q[:] = []
            while pv_q or proj_q or qkv_q:
                if not pump_one():
                    break

    nc.compile()
    return nc


# ---------------------------------------------------------------------------
# host side
# ---------------------------------------------------------------------------

_CACHE = {}


def _get_runner():
    if "runner" in _CACHE:
        return _CACHE["runner"]

    import jax
    from jax.experimental.shard_map import shard_map
    from jax.sharding import Mesh, PartitionSpec

    from concourse.bass2jax import (
        _bass_exec_p,
        install_neuronx_cc_hook,
        partition_id_tensor,
    )

    install_neuronx_cc_hook()
    nc = build_nc()
    n_cores = 8

    partition_name = nc.partition_id_tensor.name if nc.partition_id_tensor else None
    in_names = []
    out_names = []
    out_avals = []
    for alloc in nc.m.functions[0].allocations:
        if not isinstance(alloc, mybir.MemoryLocationSet):
            continue
        name = alloc.memorylocations[0].name
        if alloc.kind == "ExternalInput":
            if name != partition_name:
                in_names.append(name)
        elif alloc.kind == "ExternalOutput":
            out_names.append(name)
            out_avals.append(
                jax.core.ShapedArray(tuple(alloc.tensor_shape), mybir.dt.np(alloc.dtype))
            )
    n_params = len(in_names)
    all_names = in_names + out_names
    if partition_name is not None:
        all_names = all_names + [partition_name]

    def _body(*args):
        operands = list(args)
        if partition_name is not None:
            operands.append(partition_id_tensor())
        outs = _bass_exec_p.bind(
            *operands,
            out_avals=tuple(out_avals),
            in_names=tuple(all_names),
            out_names=tuple(out_names),
            lowering_input_output_aliases=(),
            sim_require_finite=True,
            sim_require_nnan=True,
            nc=nc,
        )
        return tuple(outs)

    devices = jax.devices()[:n_cores]
    mesh = Mesh(np.asarray(devices), ("core",))
    n_outs = len(out_names)
    fn = jax.jit(
        shard_map(
            _body,
            mesh=mesh,
            in_specs=(PartitionSpec("core"),) * (n_params + n_outs),
            out_specs=(PartitionSpec("core"),) * n_outs,
            check_rep=False,
        ),
        keep_unused=True,
    )

    runner = {
        "fn": fn,
        "in_names": in_names,
        "out_names": out_names,
        "out_avals": out_avals,
        "n_cores": n_cores,
        "jax": jax,
    }
    _CACHE["runner"] = runner
    return runner


def _pack_pairs(a):
    """[C, N] -> [128, NK2, 2, N]: row 256*k + 128*s + p -> [p, k, s]."""
    n = a.shape[1]
    return np.ascontiguousarray(
        a.reshape(NK2, 2, 128, n).transpose(2, 0, 1, 3)
    )


def _hi_lo(a):
    from ml_dtypes import float8_e4m3

    hi = a.astype(float8_e4m3)
    lo = (a - hi.astype(np.float32)).astype(float8_e4m3)
    return hi, lo


def _prepare_in_maps(x, w_attn, b_attn, w_proj, b_proj):
    x = np.asarray(x, dtype=np.float32)
    w_attn = np.asarray(w_attn, dtype=np.float32)
    b_attn = np.asarray(b_attn, dtype=np.float32)
    w_proj = np.asarray(w_proj, dtype=np.float32)
    b_proj = np.asarray(b_proj, dtype=np.float32)

    in_maps = []
    for core in range(8):
        b = core // 2
        c0 = CL * (core % 2)

        xT = np.ascontiguousarray(x[b].T) * FP8_SCALE_X  # [C, T] f32
        x_hi, x_lo = _hi_lo(xT)
        xp_h = _pack_pairs(x_hi)  # [128, 4, 2, T]
        xp_l = _pack_pairs(x_lo)

        w_all = np.concatenate(
            [
                w_attn[:, c0 : c0 + CL] * 0.125,
                w_attn[:, C + c0 : C + c0 + CL],
                w_attn[:, 2 * C + c0 : 2 * C + c0 + CL],
            ],
            axis=1,
        )  # [C, 1536] = q(512) | k(512) | v(512)
        w_all = w_all * FP8_SCALE_W
        w_hi, w_lo = _hi_lo(w_all)
        wp_h = _pack_pairs(w_hi)  # [128, 4, 2, 1536]
        wp_l = _pack_pairs(w_lo)

        def pack_first(wp, xp):
            # w cols nn0(q0:128), nn1(q128:256), nn4(k512:640), nn5(k640:768)
            # then x t0:512
            return np.ascontiguousarray(
                np.concatenate(
                    [
                        wp[:, :, :, 0:128],
                        wp[:, :, :, 128:256],
                        wp[:, :, :, 512:640],
                        wp[:, :, :, 640:768],
                        xp[:, :, :, 0:512],
                    ],
                    axis=3,
                )
            )

        def pack_wrest(wp):
            # nn2(q256:384), nn3(q384:512), nn6(k768:896), nn7(k896:1024), v
            return np.ascontiguousarray(
                np.concatenate(
                    [
                        wp[:, :, :, 256:384],
                        wp[:, :, :, 384:512],
                        wp[:, :, :, 768:896],
                        wp[:, :, :, 896:1024],
                        wp[:, :, :, 1024:1536],
                    ],
                    axis=3,
                )
            )

        first_hh = pack_first(wp_h, xp_h)
        first_ll = pack_first(wp_l, xp_l)
        wrest_hh = pack_wrest(wp_h)
        wrest_ll = pack_wrest(wp_l)
        xrest_hh = np.ascontiguousarray(xp_h[:, :, :, 512:T])
        xrest_ll = np.ascontiguousarray(xp_l[:, :, :, 512:T])

        # w_proj fp8 hi/lo, DR-packed: row 256k+128s+p -> [p, k, s, :]
        wpj = w_proj[c0 : c0 + CL, :] * FP8_SCALE_W
        wpj_hi, wpj_lo = _hi_lo(wpj)
        wpjhh = np.ascontiguousarray(
            wpj_hi.reshape(2, 2, 128, C).transpose(2, 0, 1, 3)
        )
        wpjll = np.ascontiguousarray(
            wpj_lo.reshape(2, 2, 128, C).transpose(2, 0, 1, 3)
        )

        b_q = b_attn[c0 : c0 + CL] * 0.125
        b_k = b_attn[C + c0 : C + c0 + CL]
        bqk = np.ascontiguousarray(
            np.concatenate([b_q, b_k]).reshape(8, 128).T.astype(np.float32)
        )
        b_v = b_attn[2 * C + c0 : 2 * C + c0 + CL].astype(np.float32)
        bv = np.ascontiguousarray(np.broadcast_to(b_v[None, :], (128, CL)))
        if core % 2 == 0:
            bp = np.ascontiguousarray(b_proj.reshape(8, 128).T.astype(np.float32))
        else:
            bp = np.zeros((128, 8), dtype=np.float32)

        in_maps.append(
            {
                "first_hh": first_hh,
                "first_ll": first_ll,
                "wrest_hh": wrest_hh,
                "wrest_ll": wrest_ll,
                "xrest_hh": xrest_hh,
                "xrest_ll": xrest_ll,
                "wpjhh": wpjhh,
                "wpjll": wpjll,
                "bqk": bqk,
                "bv": bv,
                "bproj": bp,
            }
        )
    return in_maps


def _run_device(in_maps):
    r = _get_runner()
    jax = r["jax"]
    n = r["n_cores"]
    per_core = [[np.asarray(m[name]) for name in r["in_names"]] for m in in_maps]
    concat_in = [
        np.concatenate([per_core[c][i] for c in range(n)], axis=0)
        for i in range(len(r["in_names"]))
    ]
    concat_zero = [
        np.zeros((n * a.shape[0], *a.shape[1:]), a.dtype) for a in r["out_avals"]
    ]
    outs = r["fn"](*[jax.device_put(a) for a in concat_in + concat_zero])
    jax.block_until_ready(outs)
    (outT,) = [np.asarray(o) for o in outs]
    return outT.reshape(n, C, T)


def kernel(x, w_attn, b_attn, w_proj, b_proj):
    in_maps = _prepare_in_maps(x, w_attn, b_attn, w_proj, b_proj)
    outT = _run_device(in_maps)
    out = np.empty((B, T, C), dtype=np.float32)
    for b in range(B):
        out[b] = (outT[2 * b].astype(np.float32) + outT[2 * b + 1].astype(np.float32)).T
    return out
